# revision 13
# baseline (speedup 1.0000x reference)
"""Trainium2 Bass kernel for DeepMinAttLSTM (4x minLSTM + MHSA + last-step FC).

Strategy:
  - Data-parallel over batch: 16 batches -> 8 cores x 2 batches.
  - Activations feature-major: X^T [H=1024 (8 tiles of 128), B*S=2048] bf16,
    plus an fp8 e4m3 copy (x*16) for the gate matmuls.
  - Per layer: f/i gate matmuls in fp8 with DoubleRow (2x PE throughput,
    W*256 / x*16 scaling, descale 1/4096 folded into the activation),
    h~ matmul in bf16. Gate math: f,i sigmoids on ACT, d=f+i on GpSimd
    (offload), r=1/d via DVE reciprocal_approx_fast, fp=f*r, and the
    input-gate branch uses ip = 1-fp exactly: add = h~' - fp*h~'.
  - Recurrence via DVE tensor_tensor_scan along time (fp32 state).
  - Attention collapses to the last query position; the full K matmul is
    eliminated via q~_bj = Wk_j^T q_bj so scores_bjk = q~_bj . h4_bk
    (K bias shifts all logits equally -> cancels in softmax). Scores, V,
    and the o-accumulation all run in fp8 DoubleRow.
  - Expected rel err ~1.2e-2 (fp8 gates dominate; threshold 2e-2).
"""

import math

import numpy as np
import ml_dtypes

BF16 = ml_dtypes.bfloat16
F8E4 = ml_dtypes.float8_e4m3

P = 128
H = 1024
S = 1024
B = 16
NCORES = 8
BC = B // NCORES          # batches per core
BS = BC * S               # 2048 free columns per core
KO = H // P               # 8 feature partition-tiles
NH = 8
DH = H // NH              # 128
O = 256
L = 4
QSCALE = 1.0 / math.sqrt(DH)
XS = 16.0                 # fp8 scale for activations
WS = 256.0                # fp8 scale for gate weights
ES = 64.0                 # fp8 scale for attention small tensors
GSC = 1.0 / (XS * WS)     # psum descale for fp8 gate matmuls

_CACHE = {}


def _build_nc():
    import concourse.mybir as mybir
    import concourse.tile as tile
    from concourse import bacc

    DT = mybir.dt.bfloat16
    F8 = mybir.dt.float8e4
    F32 = mybir.dt.float32
    AFT = mybir.ActivationFunctionType
    OP = mybir.AluOpType
    DR = mybir.MatmulPerfMode.DoubleRow

    nc = bacc.Bacc("TRN2", target_bir_lowering=False, debug=False,
                   num_devices=NCORES)

    xT8 = nc.dram_tensor("xT8", [P, KO * BS], F8, kind="ExternalInput").ap()
    xTb = nc.dram_tensor("xTb", [P, KO * BS], DT, kind="ExternalInput").ap()
    gw8 = nc.dram_tensor("gw8", [2 * L * P, KO * H], F8,
                         kind="ExternalInput").ap()
    gwh = nc.dram_tensor("gwh", [L * P, KO * H], DT, kind="ExternalInput").ap()
    gnb = nc.dram_tensor("gnb", [P, 2 * L * KO], F32,
                         kind="ExternalInput").ap()
    ghb = nc.dram_tensor("ghb", [P, L * KO], F32, kind="ExternalInput").ap()
    wq = nc.dram_tensor("wq", [P, KO * H], DT, kind="ExternalInput").ap()
    wk = nc.dram_tensor("wk", [P, NH * H], DT, kind="ExternalInput").ap()
    wv = nc.dram_tensor("wv", [P, KO * H], F8, kind="ExternalInput").ap()
    qb = nc.dram_tensor("qb", [P, KO], F32, kind="ExternalInput").ap()
    vb = nc.dram_tensor("vb", [P, NH], F32, kind="ExternalInput").ap()
    ow = nc.dram_tensor("ow", [P, KO * H], DT, kind="ExternalInput").ap()
    ob = nc.dram_tensor("ob", [P, KO], F32, kind="ExternalInput").ap()
    fcw = nc.dram_tensor("fcw", [P, KO * O], DT, kind="ExternalInput").ap()
    fcb = nc.dram_tensor("fcb", [P, O // P], F32, kind="ExternalInput").ap()
    idm = nc.dram_tensor("idm", [P, P], F32, kind="ExternalInput").ap()
    outT = nc.dram_tensor("outT", [O, BC], F32, kind="ExternalOutput").ap()

    with tile.TileContext(nc) as tc:
        with (
            tc.tile_pool(name="constp", bufs=1) as constp,
            tc.tile_pool(name="hbuf", bufs=2) as hp,
        ):
            gnb_sb = constp.tile([P, 2 * L * KO], F32)
            ghb_sb = constp.tile([P, L * KO], F32)
            nc.gpsimd.dma_start(gnb_sb[:], gnb[:])
            nc.gpsimd.dma_start(ghb_sb[:], ghb[:])

            x8p = tc.tile_pool(name="x8p", bufs=2)
            x8pool = x8p.__enter__()

            X = hp.tile([P, KO * BS], DT, tag="hbuf", name="xT_sb")
            X8 = x8pool.tile([P, KO * BS], F8, tag="x8", name="x8_0")
            xT8_v = xT8.rearrange("p (k m) -> p k m", k=KO)
            xTb_v = xTb.rearrange("p (k m) -> p k m", k=KO)
            X_v = X.rearrange("p (k m) -> p k m", k=KO)
            X8_v = X8.rearrange("p (k m) -> p k m", k=KO)
            for ch in range(4):
                m0 = ch * 512
                nc.sync.dma_start(X8_v[:, :, m0:m0 + 512],
                                  xT8_v[:, :, m0:m0 + 512])
                nc.scalar.dma_start(X_v[:, :, m0:m0 + 512],
                                    xTb_v[:, :, m0:m0 + 512])

            # ---------------- minLSTM layers ----------------
            with (
                tc.tile_pool(name="gw8p", bufs=2) as gw8p,
                tc.tile_pool(name="gwhp", bufs=2) as gwhp,
                tc.tile_pool(name="fpp", bufs=4) as fpp,
                tc.tile_pool(name="addp", bufs=4) as addp,
                tc.tile_pool(name="tmpp", bufs=4) as tmpp,
                tc.tile_pool(name="psA", bufs=8, space="PSUM") as psA,
            ):
                for l in range(L):
                    gwf = gw8p.tile([P, KO * H], F8, tag="gwf",
                                    name=f"gwf_{l}")
                    gwi = gw8p.tile([P, KO * H], F8, tag="gwi",
                                    name=f"gwi_{l}")
                    gwhb = gwhp.tile([P, KO * H], DT, tag="gwh",
                                     name=f"gwh_{l}")
                    gwf_v = gwf.rearrange("p (k h) -> p k h", k=KO)
                    gwi_v = gwi.rearrange("p (k h) -> p k h", k=KO)
                    gw8_v = gw8.rearrange("(g p) (k h) -> g p k h", p=P, k=KO)
                    # chunked loads so layer-0 matmuls start early
                    for kp in range(4):
                        nc.gpsimd.dma_start(
                            gwf_v[:, 2 * kp:2 * kp + 2, :],
                            gw8_v[2 * l, :, 2 * kp:2 * kp + 2, :])
                    for kp in range(4):
                        nc.gpsimd.dma_start(
                            gwi_v[:, 2 * kp:2 * kp + 2, :],
                            gw8_v[2 * l + 1, :, 2 * kp:2 * kp + 2, :])
                    for hf in range(2):
                        nc.gpsimd.dma_start(
                            gwhb[:, hf * 4 * H:(hf + 1) * 4 * H],
                            gwh[l * P:(l + 1) * P,
                                hf * 4 * H:(hf + 1) * 4 * H])

                    h_out = hp.tile([P, KO * BS], DT, tag="hbuf",
                                    name=f"h_{l}")
                    X8n = x8pool.tile([P, KO * BS], F8, tag="x8",
                                      name=f"x8_{l + 1}")
                    X8c_v = X8.rearrange("p (k m) -> p k m", k=KO)
                    for no in range(KO):
                        for ch in range(4):
                            m0 = ch * 512
                            b, half = ch // 2, ch % 2
                            base = no * BS + b * S
                            s0 = base + half * 512
                            psF = psA.tile([P, 512], F32, tag="ps", name="psF")
                            psI = psA.tile([P, 512], F32, tag="ps", name="psI")
                            psH = psA.tile([P, 512], F32, tag="ps", name="psH")
                            for kp in range(4):
                                nc.tensor.matmul(
                                    psF[:],
                                    gwf_v[:, 2 * kp:2 * kp + 2,
                                          no * P:(no + 1) * P],
                                    X8c_v[:, 2 * kp:2 * kp + 2, m0:m0 + 512],
                                    start=(kp == 0), stop=(kp == 3),
                                    perf_mode=DR)
                            for kp in range(4):
                                nc.tensor.matmul(
                                    psI[:],
                                    gwi_v[:, 2 * kp:2 * kp + 2,
                                          no * P:(no + 1) * P],
                                    X8c_v[:, 2 * kp:2 * kp + 2, m0:m0 + 512],
                                    start=(kp == 0), stop=(kp == 3),
                                    perf_mode=DR)
                            for ko in range(KO):
                                nc.tensor.matmul(
                                    psH[:],
                                    gwhb[:, ko * H + no * P:
                                         ko * H + (no + 1) * P],
                                    X[:, ko * BS + m0: ko * BS + m0 + 512],
                                    start=(ko == 0), stop=(ko == KO - 1))
                            bF = gnb_sb[:, (l * 2 + 0) * KO + no:
                                        (l * 2 + 0) * KO + no + 1]
                            bI = gnb_sb[:, (l * 2 + 1) * KO + no:
                                        (l * 2 + 1) * KO + no + 1]
                            bH = ghb_sb[:, l * KO + no: l * KO + no + 1]
                            f_t = tmpp.tile([P, 512], DT, tag="f", name="f_t")
                            i_t = tmpp.tile([P, 512], DT, tag="i", name="i_t")
                            d_t = tmpp.tile([P, 512], F32, tag="d", name="d_t")
                            r_t = tmpp.tile([P, 512], F32, tag="r", name="r_t")
                            ip_t = tmpp.tile([P, 512], DT, tag="ip",
                                             name="ip_t")
                            fp_t = fpp.tile([P, 512], DT, tag="fp",
                                            name="fp_t")
                            add_t = addp.tile([P, 512], DT, tag="add",
                                              name="add_t")
                            nc.scalar.activation(f_t[:], psF[:], AFT.Sigmoid,
                                                 bias=bF, scale=GSC)
                            nc.scalar.activation(i_t[:], psI[:], AFT.Sigmoid,
                                                 bias=bI, scale=GSC)
                            # d = f+i on GpSimd (DVE offload); fp = f/d;
                            # ip = 1-fp exactly; add = (zh+bh)*ip straight
                            # from PSUM (no h~ copy needed)
                            nc.gpsimd.tensor_add(d_t[:], f_t[:], i_t[:])
                            nc.vector.reciprocal_approx_fast(r_t[:], d_t[:])
                            nc.vector.tensor_mul(fp_t[:], f_t[:], r_t[:])
                            nc.vector.tensor_scalar(ip_t[:], fp_t[:], -1.0,
                                                    1.0, op0=OP.mult,
                                                    op1=OP.add)
                            nc.vector.scalar_tensor_tensor(
                                add_t[:], psH[:], bH, ip_t[:],
                                op0=OP.add, op1=OP.mult)
                            if half == 0:
                                nc.vector.tensor_tensor_scan(
                                    h_out[:, s0:s0 + 512],
                                    fp_t[:], add_t[:],
                                    initial=0.0, op0=OP.mult, op1=OP.add)
                            else:
                                nc.vector.tensor_tensor_scan(
                                    h_out[:, s0:s0 + 512],
                                    fp_t[:], add_t[:],
                                    initial=h_out[:, s0 - 1:s0],
                                    op0=OP.mult, op1=OP.add)
                            nc.scalar.activation(
                                X8n[:, s0:s0 + 512], h_out[:, s0:s0 + 512],
                                AFT.Copy, scale=XS)
                    X = h_out
                    X8 = X8n

            h4 = X
            h4_8 = X8
            h48_v = h4_8.rearrange("p (k m) -> p k m", k=KO)

            # ---------------- attention (last query position only) ----------
            with (
                tc.tile_pool(name="awp", bufs=1) as awp,
                tc.tile_pool(name="vp", bufs=1) as vp,
                tc.tile_pool(name="smallp", bufs=1) as smallp,
            ):
                wq_sb = awp.tile([P, KO * H], DT)
                nc.sync.dma_start(wq_sb[:], wq[:])
                wk_sb = awp.tile([P, NH * H], DT)
                nc.sync.dma_start(wk_sb[:], wk[:])
                wv_sb = awp.tile([P, KO * H], F8)
                nc.sync.dma_start(wv_sb[:], wv[:])
                wv_v = wv_sb.rearrange("p (k h) -> p k h", k=KO)
                ow_sb = awp.tile([P, KO * H], DT)
                nc.sync.dma_start(ow_sb[:], ow[:])
                fcw_sb = awp.tile([P, KO * O], DT)
                nc.sync.dma_start(fcw_sb[:], fcw[:])
                qb_sb = constp.tile([P, KO], F32)
                nc.gpsimd.dma_start(qb_sb[:], qb[:])
                vb_sb = constp.tile([P, NH], F32)
                nc.gpsimd.dma_start(vb_sb[:], vb[:])
                ob_sb = constp.tile([P, KO], F32)
                nc.gpsimd.dma_start(ob_sb[:], ob[:])
                fcb_sb = constp.tile([P, O // P], F32)
                nc.gpsimd.dma_start(fcb_sb[:], fcb[:])
                idm_sb = constp.tile([P, P], F32)
                nc.gpsimd.dma_start(idm_sb[:], idm[:])

                V_sb = vp.tile([P, BC * KO * H], F8, name="V_sb")
                V_v = V_sb.rearrange("p (b k h) -> p b k h", b=BC, k=KO)
                lastq = smallp.tile([P, KO * BC], DT)     # col = ko*2 + b
                q_sb = smallp.tile([P, KO * BC], DT)      # col = nt*2 + b
                qt_fm = smallp.tile([P, KO * NH * BC], F8)  # col=(dt*8+j)*2+b
                e_sb = smallp.tile([NH, BC * S], DT)      # col = b*S + s
                en_sb = smallp.tile([NH, BC * S], F32)
                acc = smallp.tile([NH, 2 * BC], F32)      # col = b*2 + ch
                den = smallp.tile([NH, BC], F32)
                rden = smallp.tile([NH, BC], F32)
                eT_sb = smallp.tile([P, BC * KO * NH], F8)  # col=(b*8+kt)*8+j
                eT_v = eT_sb.rearrange("p (b k j) -> p b k j", b=BC, k=KO)
                O_last = smallp.tile([P, NH * BC], DT)    # col = j*2 + b
                out_last = smallp.tile([P, KO * BC], DT)
                res_sb = smallp.tile([P, 2 * (O // P)], F32)

                # h4 columns at the last timestep: one strided copy
                h4_l = h4.rearrange("p (k b s) -> p k b s", k=KO, b=BC)
                lq_v = lastq.rearrange("p (k b) -> p k b", k=KO)
                nc.vector.tensor_copy(lq_v[:, :, :],
                                      h4_l[:, :, :, S - 1:S])

                with (
                    tc.tile_pool(name="psS", bufs=2, space="PSUM") as psS,
                    tc.tile_pool(name="psT", bufs=2, space="PSUM") as psT,
                    tc.tile_pool(name="psO", bufs=1, space="PSUM") as psO,
                    tc.tile_pool(name="psV", bufs=3, space="PSUM") as psV,
                ):
                    # q at the last position (scaled by QSCALE via bias+scale)
                    for nt in range(KO):
                        ps = psS.tile([P, BC], F32, tag="s", name="psq")
                        for ko in range(KO):
                            nc.tensor.matmul(
                                ps[:],
                                wq_sb[:, ko * H + nt * P:
                                      ko * H + (nt + 1) * P],
                                lastq[:, ko * BC:(ko + 1) * BC],
                                start=(ko == 0), stop=(ko == KO - 1))
                        nc.scalar.activation(
                            q_sb[:, nt * BC:(nt + 1) * BC], ps[:],
                            AFT.Identity, bias=qb_sb[:, nt:nt + 1],
                            scale=QSCALE)
                    # q~ feature-major fp8: q~[dt-tile, (j, b)] = Wk_j^T q_bj
                    for dt in range(KO):
                        ps = psS.tile([P, NH * BC], F32, tag="s", name="psqt")
                        for j in range(NH):
                            nc.tensor.matmul(
                                ps[:, j * BC:(j + 1) * BC],
                                wk_sb[:, j * H + dt * P: j * H + (dt + 1) * P],
                                q_sb[:, j * BC:(j + 1) * BC],
                                start=True, stop=True)
                        nc.scalar.activation(
                            qt_fm[:, dt * NH * BC:(dt + 1) * NH * BC], ps[:],
                            AFT.Copy, scale=ES)
                    # scores [8 heads, 512 pos] per (b, ch) via fp8 DR + exp
                    qt_v = qt_fm.rearrange("p (k j b) -> p k j b", k=KO, j=NH)
                    for b in range(BC):
                        for ch in range(2):
                            m0 = b * S + ch * 512
                            ps = psS.tile([NH, 512], F32, tag="s",
                                          name="pssc")
                            for kp in range(4):
                                nc.tensor.matmul(
                                    ps[:],
                                    qt_v[:, 2 * kp:2 * kp + 2, :, b:b + 1],
                                    h48_v[:, 2 * kp:2 * kp + 2, m0:m0 + 512],
                                    start=(kp == 0), stop=(kp == 3),
                                    perf_mode=DR)
                            nc.scalar.activation(
                                e_sb[:, m0:m0 + 512], ps[:], AFT.Exp,
                                scale=1.0 / (ES * XS),
                                accum_out=acc[:, b * 2 + ch:b * 2 + ch + 1])
                    # denominators and normalized weights (fp8, *ES)
                    acc_v = acc.rearrange("p (b c) -> p b c", b=BC)
                    nc.vector.tensor_add(den[:, :], acc_v[:, :, 0:1],
                                         acc_v[:, :, 1:2])
                    nc.vector.reciprocal_approx_fast(rden[:, :], den[:, :])
                    for b in range(BC):
                        nc.scalar.activation(
                            en_sb[:, b * S:(b + 1) * S],
                            e_sb[:, b * S:(b + 1) * S],
                            AFT.Copy, scale=rden[:, b:b + 1])
                    # transpose normalized weights to [pos, head], cast fp8
                    for b in range(BC):
                        for kt in range(KO):
                            pst = psT.tile([P, NH], F32, tag="t", name="pst")
                            nc.tensor.transpose(
                                pst[:],
                                en_sb[:, b * S + kt * P: b * S + (kt + 1) * P],
                                idm_sb[0:NH, 0:NH])
                            nc.scalar.activation(
                                eT_sb[:, (b * KO + kt) * NH:
                                      (b * KO + kt + 1) * NH],
                                pst[:], AFT.Copy, scale=ES)

                    # V position-major [BS, H] via fp8 DoubleRow, stored fp8
                    for st in range(BC * KO):
                        b, si = st // KO, st % KO
                        m0 = b * S + si * P
                        for dch in range(2):
                            d0 = dch * 512
                            ps = psV.tile([P, 512], F32, tag="v", name="psv")
                            for kp in range(4):
                                nc.tensor.matmul(
                                    ps[:],
                                    h48_v[:, 2 * kp:2 * kp + 2, m0:m0 + P],
                                    wv_v[:, 2 * kp:2 * kp + 2, d0:d0 + 512],
                                    start=(kp == 0), stop=(kp == 3),
                                    perf_mode=DR)
                            nc.scalar.activation(
                                V_sb[:, st * H + d0: st * H + d0 + 512],
                                ps[:], AFT.Copy, scale=GSC * ES)

                    # o at last position via fp8 DoubleRow over position tiles
                    ps_ob = psO.tile([P, NH * BC], F32, tag="o", name="ps_ob")
                    for b in range(BC):
                        for j in range(NH):
                            c = j * BC + b
                            for kp in range(4):
                                nc.tensor.matmul(
                                    ps_ob[:, c:c + 1],
                                    V_v[:, b, 2 * kp:2 * kp + 2,
                                        j * P:(j + 1) * P],
                                    eT_v[:, b, 2 * kp:2 * kp + 2, j:j + 1],
                                    start=(kp == 0), stop=(kp == 3),
                                    perf_mode=DR)
                    for j in range(NH):
                        nc.scalar.activation(
                            O_last[:, j * BC:(j + 1) * BC],
                            ps_ob[:, j * BC:(j + 1) * BC],
                            AFT.Identity, bias=vb_sb[:, j:j + 1],
                            scale=1.0 / (ES * ES))
                    # out projection + residual
                    for no in range(KO):
                        ps = psS.tile([P, BC], F32, tag="s", name="psp")
                        for ko in range(KO):
                            nc.tensor.matmul(
                                ps[:],
                                ow_sb[:, ko * H + no * P: ko * H + (no + 1) * P],
                                O_last[:, ko * BC:(ko + 1) * BC],
                                start=(ko == 0), stop=(ko == KO - 1))
                        nc.vector.scalar_tensor_tensor(
                            out_last[:, no * BC:(no + 1) * BC],
                            ps[:], ob_sb[:, no:no + 1],
                            lastq[:, no * BC:(no + 1) * BC],
                            op0=OP.add, op1=OP.add)
                    # final fc
                    for ot in range(O // P):
                        ps = psS.tile([P, BC], F32, tag="s", name="psf")
                        for ko in range(KO):
                            nc.tensor.matmul(
                                ps[:],
                                fcw_sb[:, ko * O + ot * P: ko * O + (ot + 1) * P],
                                out_last[:, ko * BC:(ko + 1) * BC],
                                start=(ko == 0), stop=(ko == KO - 1))
                        nc.scalar.activation(
                            res_sb[:, ot * BC:(ot + 1) * BC], ps[:],
                            AFT.Identity, bias=fcb_sb[:, ot:ot + 1])
                        nc.sync.dma_start(
                            outT[ot * P:(ot + 1) * P, :],
                            res_sb[:, ot * BC:(ot + 1) * BC])

            x8p.__exit__(None, None, None)

    nc.compile()
    return nc


def _feature_major(w_t):
    """[H_in, N] (already transposed weight) -> device layout [128, KO*N]."""
    hin, n = w_t.shape
    ko = hin // P
    return np.ascontiguousarray(
        w_t.reshape(ko, P, n).transpose(1, 0, 2).reshape(P, ko * n))


def _prep_inputs(x, Wf, bf, Wi, bi, Wh, bh, in_proj_w, in_proj_b, out_w,
                 out_b, fc_w, fc_b):
    gw8s, gwhs, gnbs, ghbs = [], [], [], []
    for l in range(L):
        for W, bias in ((Wf[l], bf[l]), (Wi[l], bi[l])):
            fm = _feature_major(W.T.astype(np.float32) * WS)
            gw8s.append(fm.astype(F8E4))
            gnbs.append(bias.reshape(KO, P).T.astype(np.float32))
        gwhs.append(_feature_major(Wh[l].T.astype(np.float32)).astype(BF16))
        ghbs.append(bh[l].reshape(KO, P).T.astype(np.float32))
    gw8 = np.concatenate(gw8s, axis=0)                   # [2L*128, KO*H]
    gwh = np.concatenate(gwhs, axis=0)                   # [L*128, KO*H]
    gnb = np.ascontiguousarray(np.concatenate(gnbs, axis=1))
    ghb = np.ascontiguousarray(np.concatenate(ghbs, axis=1))
    ipw = in_proj_w.astype(np.float32)
    wq_ = _feature_major(ipw[:H].T).astype(BF16)
    wk_ = np.ascontiguousarray(
        ipw[H:2 * H].reshape(NH, P, H).transpose(1, 0, 2).reshape(P, NH * H)
    ).astype(BF16)
    wv_ = (_feature_major(ipw[2 * H:].T) * WS).astype(F8E4)
    qb_ = np.ascontiguousarray(
        (in_proj_b[:H] * QSCALE).reshape(KO, P).T.astype(np.float32))
    vb_ = np.ascontiguousarray(
        in_proj_b[2 * H:].reshape(NH, P).T.astype(np.float32))
    ow_ = _feature_major(out_w.T.astype(np.float32)).astype(BF16)
    ob_ = np.ascontiguousarray(out_b.reshape(KO, P).T.astype(np.float32))
    fcw_ = _feature_major(fc_w.T.astype(np.float32)).astype(BF16)
    fcb_ = np.ascontiguousarray(
        fc_b.reshape(O // P, P).T.astype(np.float32))
    idm_ = np.eye(P, dtype=np.float32)
    shared = dict(gw8=gw8, gwh=gwh, gnb=gnb, ghb=ghb, wq=wq_, wk=wk_, wv=wv_,
                  qb=qb_, vb=vb_, ow=ow_, ob=ob_, fcw=fcw_, fcb=fcb_,
                  idm=idm_)
    in_maps = []
    for c in range(NCORES):
        shard = x[c * BC:(c + 1) * BC]                   # [BC, S, H]
        xt = shard.transpose(2, 0, 1).reshape(H, BS)     # [H, BS]
        xt = _feature_major(xt.astype(np.float32))       # [128, KO*BS]
        in_maps.append(dict(shared, xT8=(xt * XS).astype(F8E4),
                            xTb=xt.astype(BF16)))
    return in_maps


def kernel(x, Wf, bf, Wi, bi, Wh, bh, in_proj_w, in_proj_b, out_w, out_b,
           fc_w, fc_b):
    from concourse.bass_utils import run_bass_kernel_spmd

    x, Wf, bf, Wi, bi, Wh, bh = (np.asarray(t) for t in
                                 (x, Wf, bf, Wi, bi, Wh, bh))
    in_proj_w, in_proj_b, out_w, out_b, fc_w, fc_b = (
        np.asarray(t) for t in (in_proj_w, in_proj_b, out_w, out_b,
                                fc_w, fc_b))
    if "nc" not in _CACHE:
        _CACHE["nc"] = _build_nc()
    nc = _CACHE["nc"]
    in_maps = _prep_inputs(x, Wf, bf, Wi, bi, Wh, bh, in_proj_w, in_proj_b,
                           out_w, out_b, fc_w, fc_b)
    res = run_bass_kernel_spmd(nc, in_maps, core_ids=list(range(NCORES)))
    _CACHE["last_results"] = res
    out = np.empty((B, O), np.float32)
    for c in range(NCORES):
        outT = res.results[c]["outT"]                    # [O, BC]
        for b in range(BC):
            out[c * BC + b] = outT[:, b]
    return out


# revision 14
# speedup vs baseline: 1.0101x; 1.0101x over previous
"""Trainium2 Bass kernel for DeepMinAttLSTM (4x minLSTM + MHSA + last-step FC).

Strategy:
  - Data-parallel over batch: 16 batches -> 8 cores x 2 batches.
  - Activations feature-major: X^T [H=1024 (8 tiles of 128), B*S=2048] bf16,
    plus an fp8 e4m3 copy (x*16) for the gate matmuls.
  - Per layer: f/i gate matmuls in fp8 with DoubleRow (2x PE throughput,
    W*256 / x*16 scaling, descale 1/4096 folded into the activation),
    h~ matmul in bf16. Gate math: f,i sigmoids on ACT, d=f+i on GpSimd
    (offload), r=1/d via DVE reciprocal_approx_fast, fp=f*r, and the
    input-gate branch uses ip = 1-fp exactly: add = h~' - fp*h~'.
  - Recurrence via DVE tensor_tensor_scan along time (fp32 state).
  - Attention collapses to the last query position; the full K matmul is
    eliminated via q~_bj = Wk_j^T q_bj so scores_bjk = q~_bj . h4_bk
    (K bias shifts all logits equally -> cancels in softmax). Scores, V,
    and the o-accumulation all run in fp8 DoubleRow.
  - Expected rel err ~1.2e-2 (fp8 gates dominate; threshold 2e-2).
"""

import math

import numpy as np
import ml_dtypes

BF16 = ml_dtypes.bfloat16
F8E4 = ml_dtypes.float8_e4m3

P = 128
H = 1024
S = 1024
B = 16
NCORES = 8
BC = B // NCORES          # batches per core
BS = BC * S               # 2048 free columns per core
KO = H // P               # 8 feature partition-tiles
NH = 8
DH = H // NH              # 128
O = 256
L = 4
QSCALE = 1.0 / math.sqrt(DH)
XS = 16.0                 # fp8 scale for activations
WS = 256.0                # fp8 scale for gate weights
ES = 64.0                 # fp8 scale for attention small tensors
GSC = 1.0 / (XS * WS)     # psum descale for fp8 gate matmuls

_CACHE = {}


def _build_nc():
    import concourse.mybir as mybir
    import concourse.tile as tile
    from concourse import bacc

    DT = mybir.dt.bfloat16
    F8 = mybir.dt.float8e4
    F32 = mybir.dt.float32
    AFT = mybir.ActivationFunctionType
    OP = mybir.AluOpType
    DR = mybir.MatmulPerfMode.DoubleRow

    nc = bacc.Bacc("TRN2", target_bir_lowering=False, debug=False,
                   num_devices=NCORES)

    xT8 = nc.dram_tensor("xT8", [P, KO * BS], F8, kind="ExternalInput").ap()
    xTb = nc.dram_tensor("xTb", [P, KO * BS], DT, kind="ExternalInput").ap()
    gw8 = nc.dram_tensor("gw8", [2 * L * P, KO * H], F8,
                         kind="ExternalInput").ap()
    gwh = nc.dram_tensor("gwh", [L * P, KO * H], DT, kind="ExternalInput").ap()
    gnb = nc.dram_tensor("gnb", [P, 2 * L * KO], F32,
                         kind="ExternalInput").ap()
    ghb = nc.dram_tensor("ghb", [P, L * KO], F32, kind="ExternalInput").ap()
    wq = nc.dram_tensor("wq", [P, KO * H], DT, kind="ExternalInput").ap()
    wk = nc.dram_tensor("wk", [P, NH * H], DT, kind="ExternalInput").ap()
    wv = nc.dram_tensor("wv", [P, KO * H], F8, kind="ExternalInput").ap()
    qb = nc.dram_tensor("qb", [P, KO], F32, kind="ExternalInput").ap()
    vb = nc.dram_tensor("vb", [P, NH], F32, kind="ExternalInput").ap()
    ow = nc.dram_tensor("ow", [P, KO * H], DT, kind="ExternalInput").ap()
    ob = nc.dram_tensor("ob", [P, KO], F32, kind="ExternalInput").ap()
    fcw = nc.dram_tensor("fcw", [P, KO * O], DT, kind="ExternalInput").ap()
    fcb = nc.dram_tensor("fcb", [P, O // P], F32, kind="ExternalInput").ap()
    idm = nc.dram_tensor("idm", [P, P], F32, kind="ExternalInput").ap()
    outT = nc.dram_tensor("outT", [O, BC], F32, kind="ExternalOutput").ap()

    with tile.TileContext(nc) as tc:
        with (
            tc.tile_pool(name="constp", bufs=1) as constp,
            tc.tile_pool(name="hbuf", bufs=2) as hp,
        ):
            gnb_sb = constp.tile([P, 2 * L * KO], F32)
            ghb_sb = constp.tile([P, L * KO], F32)
            nc.gpsimd.dma_start(gnb_sb[:], gnb[:])
            nc.gpsimd.dma_start(ghb_sb[:], ghb[:])

            x8p = tc.tile_pool(name="x8p", bufs=2)
            x8pool = x8p.__enter__()

            X = hp.tile([P, KO * BS], DT, tag="hbuf", name="xT_sb")
            X8 = x8pool.tile([P, KO * BS], F8, tag="x8", name="x8_0")
            xT8_v = xT8.rearrange("p (k m) -> p k m", k=KO)
            xTb_v = xTb.rearrange("p (k m) -> p k m", k=KO)
            X_v = X.rearrange("p (k m) -> p k m", k=KO)
            X8_v = X8.rearrange("p (k m) -> p k m", k=KO)
            for ch in range(4):
                m0 = ch * 512
                nc.sync.dma_start(X8_v[:, :, m0:m0 + 512],
                                  xT8_v[:, :, m0:m0 + 512])
                nc.scalar.dma_start(X_v[:, :, m0:m0 + 512],
                                    xTb_v[:, :, m0:m0 + 512])

            # ---------------- minLSTM layers ----------------
            with (
                tc.tile_pool(name="gw8p", bufs=2) as gw8p,
                tc.tile_pool(name="gwhp", bufs=2) as gwhp,
                tc.tile_pool(name="fpp", bufs=3) as fpp,
                tc.tile_pool(name="addp", bufs=3) as addp,
                tc.tile_pool(name="tmpp", bufs=3) as tmpp,
                tc.tile_pool(name="psA", bufs=6, space="PSUM") as psA,
            ):
                for l in range(L):
                    gwf = gw8p.tile([P, KO * H], F8, tag="gwf",
                                    name=f"gwf_{l}")
                    gwi = gw8p.tile([P, KO * H], F8, tag="gwi",
                                    name=f"gwi_{l}")
                    gwhb = gwhp.tile([P, KO * H], DT, tag="gwh",
                                     name=f"gwh_{l}")
                    gwf_v = gwf.rearrange("p (k h) -> p k h", k=KO)
                    gwi_v = gwi.rearrange("p (k h) -> p k h", k=KO)
                    gw8_v = gw8.rearrange("(g p) (k h) -> g p k h", p=P, k=KO)
                    # chunked loads so layer-0 matmuls start early
                    for kp in range(4):
                        nc.gpsimd.dma_start(
                            gwf_v[:, 2 * kp:2 * kp + 2, :],
                            gw8_v[2 * l, :, 2 * kp:2 * kp + 2, :])
                    for kp in range(4):
                        nc.gpsimd.dma_start(
                            gwi_v[:, 2 * kp:2 * kp + 2, :],
                            gw8_v[2 * l + 1, :, 2 * kp:2 * kp + 2, :])
                    for hf in range(2):
                        nc.gpsimd.dma_start(
                            gwhb[:, hf * 4 * H:(hf + 1) * 4 * H],
                            gwh[l * P:(l + 1) * P,
                                hf * 4 * H:(hf + 1) * 4 * H])

                    h_out = hp.tile([P, KO * BS], DT, tag="hbuf",
                                    name=f"h_{l}")
                    X8n = x8pool.tile([P, KO * BS], F8, tag="x8",
                                      name=f"x8_{l + 1}")
                    X8c_v = X8.rearrange("p (k m) -> p k m", k=KO)
                    for no in range(KO):
                        for ch in range(4):
                            m0 = ch * 512
                            b, half = ch // 2, ch % 2
                            base = no * BS + b * S
                            s0 = base + half * 512
                            psF = psA.tile([P, 512], F32, tag="ps", name="psF")
                            psI = psA.tile([P, 512], F32, tag="ps", name="psI")
                            psH = psA.tile([P, 512], F32, tag="ps", name="psH")
                            for kp in range(4):
                                nc.tensor.matmul(
                                    psF[:],
                                    gwf_v[:, 2 * kp:2 * kp + 2,
                                          no * P:(no + 1) * P],
                                    X8c_v[:, 2 * kp:2 * kp + 2, m0:m0 + 512],
                                    start=(kp == 0), stop=(kp == 3),
                                    perf_mode=DR)
                            for kp in range(4):
                                nc.tensor.matmul(
                                    psI[:],
                                    gwi_v[:, 2 * kp:2 * kp + 2,
                                          no * P:(no + 1) * P],
                                    X8c_v[:, 2 * kp:2 * kp + 2, m0:m0 + 512],
                                    start=(kp == 0), stop=(kp == 3),
                                    perf_mode=DR)
                            for ko in range(KO):
                                nc.tensor.matmul(
                                    psH[:],
                                    gwhb[:, ko * H + no * P:
                                         ko * H + (no + 1) * P],
                                    X[:, ko * BS + m0: ko * BS + m0 + 512],
                                    start=(ko == 0), stop=(ko == KO - 1))
                            bF = gnb_sb[:, (l * 2 + 0) * KO + no:
                                        (l * 2 + 0) * KO + no + 1]
                            bI = gnb_sb[:, (l * 2 + 1) * KO + no:
                                        (l * 2 + 1) * KO + no + 1]
                            bH = ghb_sb[:, l * KO + no: l * KO + no + 1]
                            f_t = tmpp.tile([P, 512], DT, tag="f", name="f_t")
                            i_t = tmpp.tile([P, 512], DT, tag="i", name="i_t")
                            d_t = tmpp.tile([P, 512], F32, tag="d", name="d_t")
                            r_t = tmpp.tile([P, 512], F32, tag="r", name="r_t")
                            ip_t = tmpp.tile([P, 512], DT, tag="ip",
                                             name="ip_t")
                            fp_t = fpp.tile([P, 512], DT, tag="fp",
                                            name="fp_t")
                            add_t = addp.tile([P, 512], DT, tag="add",
                                              name="add_t")
                            nc.scalar.activation(f_t[:], psF[:], AFT.Sigmoid,
                                                 bias=bF, scale=GSC)
                            nc.scalar.activation(i_t[:], psI[:], AFT.Sigmoid,
                                                 bias=bI, scale=GSC)
                            # d = f+i on GpSimd (DVE offload); fp = f/d;
                            # ip = 1-fp exactly; add = (zh+bh)*ip straight
                            # from PSUM (no h~ copy needed)
                            nc.gpsimd.tensor_add(d_t[:], f_t[:], i_t[:])
                            nc.vector.reciprocal_approx_fast(r_t[:], d_t[:])
                            nc.vector.tensor_mul(fp_t[:], f_t[:], r_t[:])
                            nc.vector.tensor_scalar(ip_t[:], fp_t[:], -1.0,
                                                    1.0, op0=OP.mult,
                                                    op1=OP.add)
                            nc.vector.scalar_tensor_tensor(
                                add_t[:], psH[:], bH, ip_t[:],
                                op0=OP.add, op1=OP.mult)
                            if half == 0:
                                nc.vector.tensor_tensor_scan(
                                    h_out[:, s0:s0 + 512],
                                    fp_t[:], add_t[:],
                                    initial=0.0, op0=OP.mult, op1=OP.add)
                            else:
                                nc.vector.tensor_tensor_scan(
                                    h_out[:, s0:s0 + 512],
                                    fp_t[:], add_t[:],
                                    initial=h_out[:, s0 - 1:s0],
                                    op0=OP.mult, op1=OP.add)
                            nc.scalar.activation(
                                X8n[:, s0:s0 + 512], h_out[:, s0:s0 + 512],
                                AFT.Copy, scale=XS)
                    X = h_out
                    X8 = X8n

            h4 = X
            h4_8 = X8
            h48_v = h4_8.rearrange("p (k m) -> p k m", k=KO)

            # ---------------- attention (last query position only) ----------
            with (
                tc.tile_pool(name="awp", bufs=1) as awp,
                tc.tile_pool(name="vp", bufs=1) as vp,
                tc.tile_pool(name="smallp", bufs=1) as smallp,
            ):
                wq_sb = awp.tile([P, KO * H], DT)
                nc.sync.dma_start(wq_sb[:], wq[:])
                wk_sb = awp.tile([P, NH * H], DT)
                nc.sync.dma_start(wk_sb[:], wk[:])
                wv_sb = awp.tile([P, KO * H], F8)
                nc.sync.dma_start(wv_sb[:], wv[:])
                wv_v = wv_sb.rearrange("p (k h) -> p k h", k=KO)
                ow_sb = awp.tile([P, KO * H], DT)
                nc.sync.dma_start(ow_sb[:], ow[:])
                fcw_sb = awp.tile([P, KO * O], DT)
                nc.sync.dma_start(fcw_sb[:], fcw[:])
                qb_sb = constp.tile([P, KO], F32)
                nc.gpsimd.dma_start(qb_sb[:], qb[:])
                vb_sb = constp.tile([P, NH], F32)
                nc.gpsimd.dma_start(vb_sb[:], vb[:])
                ob_sb = constp.tile([P, KO], F32)
                nc.gpsimd.dma_start(ob_sb[:], ob[:])
                fcb_sb = constp.tile([P, O // P], F32)
                nc.gpsimd.dma_start(fcb_sb[:], fcb[:])
                idm_sb = constp.tile([P, P], F32)
                nc.gpsimd.dma_start(idm_sb[:], idm[:])

                V_sb = vp.tile([P, BC * KO * H], F8, name="V_sb")
                V_v = V_sb.rearrange("p (b k h) -> p b k h", b=BC, k=KO)
                lastq = smallp.tile([P, KO * BC], DT)     # col = ko*2 + b
                q_sb = smallp.tile([P, KO * BC], DT)      # col = nt*2 + b
                qt_fm = smallp.tile([P, KO * NH * BC], F8)  # col=(dt*8+j)*2+b
                e_sb = smallp.tile([NH, BC * S], DT)      # col = b*S + s
                en_sb = smallp.tile([NH, BC * S], F32)
                acc = smallp.tile([NH, 2 * BC], F32)      # col = b*2 + ch
                den = smallp.tile([NH, BC], F32)
                rden = smallp.tile([NH, BC], F32)
                eT_sb = smallp.tile([P, BC * KO * NH], F8)  # col=(b*8+kt)*8+j
                eT_v = eT_sb.rearrange("p (b k j) -> p b k j", b=BC, k=KO)
                O_last = smallp.tile([P, NH * BC], DT)    # col = j*2 + b
                out_last = smallp.tile([P, KO * BC], DT)
                res_sb = smallp.tile([P, 2 * (O // P)], F32)

                # h4 columns at the last timestep: one strided copy
                h4_l = h4.rearrange("p (k b s) -> p k b s", k=KO, b=BC)
                lq_v = lastq.rearrange("p (k b) -> p k b", k=KO)
                nc.vector.tensor_copy(lq_v[:, :, :],
                                      h4_l[:, :, :, S - 1:S])

                with (
                    tc.tile_pool(name="psS", bufs=2, space="PSUM") as psS,
                    tc.tile_pool(name="psT", bufs=2, space="PSUM") as psT,
                    tc.tile_pool(name="psO", bufs=1, space="PSUM") as psO,
                    tc.tile_pool(name="psV", bufs=3, space="PSUM") as psV,
                ):
                    # V position-major [BS, H] via fp8 DoubleRow, stored fp8
                    for st in range(BC * KO):
                        b, si = st // KO, st % KO
                        m0 = b * S + si * P
                        for dch in range(2):
                            d0 = dch * 512
                            ps = psV.tile([P, 512], F32, tag="v", name="psv")
                            for kp in range(4):
                                nc.tensor.matmul(
                                    ps[:],
                                    h48_v[:, 2 * kp:2 * kp + 2, m0:m0 + P],
                                    wv_v[:, 2 * kp:2 * kp + 2, d0:d0 + 512],
                                    start=(kp == 0), stop=(kp == 3),
                                    perf_mode=DR)
                            nc.scalar.activation(
                                V_sb[:, st * H + d0: st * H + d0 + 512],
                                ps[:], AFT.Copy, scale=GSC * ES)

                    # q at the last position (scaled by QSCALE via bias+scale)
                    for nt in range(KO):
                        ps = psS.tile([P, BC], F32, tag="s", name="psq")
                        for ko in range(KO):
                            nc.tensor.matmul(
                                ps[:],
                                wq_sb[:, ko * H + nt * P:
                                      ko * H + (nt + 1) * P],
                                lastq[:, ko * BC:(ko + 1) * BC],
                                start=(ko == 0), stop=(ko == KO - 1))
                        nc.scalar.activation(
                            q_sb[:, nt * BC:(nt + 1) * BC], ps[:],
                            AFT.Identity, bias=qb_sb[:, nt:nt + 1],
                            scale=QSCALE)
                    # q~ feature-major fp8: q~[dt-tile, (j, b)] = Wk_j^T q_bj
                    for dt in range(KO):
                        ps = psS.tile([P, NH * BC], F32, tag="s", name="psqt")
                        for j in range(NH):
                            nc.tensor.matmul(
                                ps[:, j * BC:(j + 1) * BC],
                                wk_sb[:, j * H + dt * P: j * H + (dt + 1) * P],
                                q_sb[:, j * BC:(j + 1) * BC],
                                start=True, stop=True)
                        nc.scalar.activation(
                            qt_fm[:, dt * NH * BC:(dt + 1) * NH * BC], ps[:],
                            AFT.Copy, scale=ES)
                    # scores [8 heads, 512 pos] per (b, ch) via fp8 DR + exp
                    qt_v = qt_fm.rearrange("p (k j b) -> p k j b", k=KO, j=NH)
                    for b in range(BC):
                        for ch in range(2):
                            m0 = b * S + ch * 512
                            ps = psS.tile([NH, 512], F32, tag="s",
                                          name="pssc")
                            for kp in range(4):
                                nc.tensor.matmul(
                                    ps[:],
                                    qt_v[:, 2 * kp:2 * kp + 2, :, b:b + 1],
                                    h48_v[:, 2 * kp:2 * kp + 2, m0:m0 + 512],
                                    start=(kp == 0), stop=(kp == 3),
                                    perf_mode=DR)
                            nc.scalar.activation(
                                e_sb[:, m0:m0 + 512], ps[:], AFT.Exp,
                                scale=1.0 / (ES * XS),
                                accum_out=acc[:, b * 2 + ch:b * 2 + ch + 1])
                    # denominators and normalized weights (fp8, *ES)
                    acc_v = acc.rearrange("p (b c) -> p b c", b=BC)
                    nc.vector.tensor_add(den[:, :], acc_v[:, :, 0:1],
                                         acc_v[:, :, 1:2])
                    nc.vector.reciprocal_approx_fast(rden[:, :], den[:, :])
                    for b in range(BC):
                        nc.scalar.activation(
                            en_sb[:, b * S:(b + 1) * S],
                            e_sb[:, b * S:(b + 1) * S],
                            AFT.Copy, scale=rden[:, b:b + 1])
                    # transpose normalized weights to [pos, head], cast fp8
                    for b in range(BC):
                        for kt in range(KO):
                            pst = psT.tile([P, NH], F32, tag="t", name="pst")
                            nc.tensor.transpose(
                                pst[:],
                                en_sb[:, b * S + kt * P: b * S + (kt + 1) * P],
                                idm_sb[0:NH, 0:NH])
                            nc.scalar.activation(
                                eT_sb[:, (b * KO + kt) * NH:
                                      (b * KO + kt + 1) * NH],
                                pst[:], AFT.Copy, scale=ES)

                    # o at last position via fp8 DoubleRow over position tiles
                    ps_ob = psO.tile([P, NH * BC], F32, tag="o", name="ps_ob")
                    for b in range(BC):
                        for j in range(NH):
                            c = j * BC + b
                            for kp in range(4):
                                nc.tensor.matmul(
                                    ps_ob[:, c:c + 1],
                                    V_v[:, b, 2 * kp:2 * kp + 2,
                                        j * P:(j + 1) * P],
                                    eT_v[:, b, 2 * kp:2 * kp + 2, j:j + 1],
                                    start=(kp == 0), stop=(kp == 3),
                                    perf_mode=DR)
                    for j in range(NH):
                        nc.scalar.activation(
                            O_last[:, j * BC:(j + 1) * BC],
                            ps_ob[:, j * BC:(j + 1) * BC],
                            AFT.Identity, bias=vb_sb[:, j:j + 1],
                            scale=1.0 / (ES * ES))
                    # out projection + residual
                    for no in range(KO):
                        ps = psS.tile([P, BC], F32, tag="s", name="psp")
                        for ko in range(KO):
                            nc.tensor.matmul(
                                ps[:],
                                ow_sb[:, ko * H + no * P: ko * H + (no + 1) * P],
                                O_last[:, ko * BC:(ko + 1) * BC],
                                start=(ko == 0), stop=(ko == KO - 1))
                        nc.vector.scalar_tensor_tensor(
                            out_last[:, no * BC:(no + 1) * BC],
                            ps[:], ob_sb[:, no:no + 1],
                            lastq[:, no * BC:(no + 1) * BC],
                            op0=OP.add, op1=OP.add)
                    # final fc
                    for ot in range(O // P):
                        ps = psS.tile([P, BC], F32, tag="s", name="psf")
                        for ko in range(KO):
                            nc.tensor.matmul(
                                ps[:],
                                fcw_sb[:, ko * O + ot * P: ko * O + (ot + 1) * P],
                                out_last[:, ko * BC:(ko + 1) * BC],
                                start=(ko == 0), stop=(ko == KO - 1))
                        nc.scalar.activation(
                            res_sb[:, ot * BC:(ot + 1) * BC], ps[:],
                            AFT.Identity, bias=fcb_sb[:, ot:ot + 1])
                        nc.sync.dma_start(
                            outT[ot * P:(ot + 1) * P, :],
                            res_sb[:, ot * BC:(ot + 1) * BC])

            x8p.__exit__(None, None, None)

    nc.compile()
    return nc


def _feature_major(w_t):
    """[H_in, N] (already transposed weight) -> device layout [128, KO*N]."""
    hin, n = w_t.shape
    ko = hin // P
    return np.ascontiguousarray(
        w_t.reshape(ko, P, n).transpose(1, 0, 2).reshape(P, ko * n))


def _prep_inputs(x, Wf, bf, Wi, bi, Wh, bh, in_proj_w, in_proj_b, out_w,
                 out_b, fc_w, fc_b):
    gw8s, gwhs, gnbs, ghbs = [], [], [], []
    for l in range(L):
        for W, bias in ((Wf[l], bf[l]), (Wi[l], bi[l])):
            fm = _feature_major(W.T.astype(np.float32) * WS)
            gw8s.append(fm.astype(F8E4))
            gnbs.append(bias.reshape(KO, P).T.astype(np.float32))
        gwhs.append(_feature_major(Wh[l].T.astype(np.float32)).astype(BF16))
        ghbs.append(bh[l].reshape(KO, P).T.astype(np.float32))
    gw8 = np.concatenate(gw8s, axis=0)                   # [2L*128, KO*H]
    gwh = np.concatenate(gwhs, axis=0)                   # [L*128, KO*H]
    gnb = np.ascontiguousarray(np.concatenate(gnbs, axis=1))
    ghb = np.ascontiguousarray(np.concatenate(ghbs, axis=1))
    ipw = in_proj_w.astype(np.float32)
    wq_ = _feature_major(ipw[:H].T).astype(BF16)
    wk_ = np.ascontiguousarray(
        ipw[H:2 * H].reshape(NH, P, H).transpose(1, 0, 2).reshape(P, NH * H)
    ).astype(BF16)
    wv_ = (_feature_major(ipw[2 * H:].T) * WS).astype(F8E4)
    qb_ = np.ascontiguousarray(
        (in_proj_b[:H] * QSCALE).reshape(KO, P).T.astype(np.float32))
    vb_ = np.ascontiguousarray(
        in_proj_b[2 * H:].reshape(NH, P).T.astype(np.float32))
    ow_ = _feature_major(out_w.T.astype(np.float32)).astype(BF16)
    ob_ = np.ascontiguousarray(out_b.reshape(KO, P).T.astype(np.float32))
    fcw_ = _feature_major(fc_w.T.astype(np.float32)).astype(BF16)
    fcb_ = np.ascontiguousarray(
        fc_b.reshape(O // P, P).T.astype(np.float32))
    idm_ = np.eye(P, dtype=np.float32)
    shared = dict(gw8=gw8, gwh=gwh, gnb=gnb, ghb=ghb, wq=wq_, wk=wk_, wv=wv_,
                  qb=qb_, vb=vb_, ow=ow_, ob=ob_, fcw=fcw_, fcb=fcb_,
                  idm=idm_)
    in_maps = []
    for c in range(NCORES):
        shard = x[c * BC:(c + 1) * BC]                   # [BC, S, H]
        xt = shard.transpose(2, 0, 1).reshape(H, BS)     # [H, BS]
        xt = _feature_major(xt.astype(np.float32))       # [128, KO*BS]
        in_maps.append(dict(shared, xT8=(xt * XS).astype(F8E4),
                            xTb=xt.astype(BF16)))
    return in_maps


def kernel(x, Wf, bf, Wi, bi, Wh, bh, in_proj_w, in_proj_b, out_w, out_b,
           fc_w, fc_b):
    from concourse.bass_utils import run_bass_kernel_spmd

    x, Wf, bf, Wi, bi, Wh, bh = (np.asarray(t) for t in
                                 (x, Wf, bf, Wi, bi, Wh, bh))
    in_proj_w, in_proj_b, out_w, out_b, fc_w, fc_b = (
        np.asarray(t) for t in (in_proj_w, in_proj_b, out_w, out_b,
                                fc_w, fc_b))
    if "nc" not in _CACHE:
        _CACHE["nc"] = _build_nc()
    nc = _CACHE["nc"]
    in_maps = _prep_inputs(x, Wf, bf, Wi, bi, Wh, bh, in_proj_w, in_proj_b,
                           out_w, out_b, fc_w, fc_b)
    res = run_bass_kernel_spmd(nc, in_maps, core_ids=list(range(NCORES)))
    _CACHE["last_results"] = res
    out = np.empty((B, O), np.float32)
    for c in range(NCORES):
        outT = res.results[c]["outT"]                    # [O, BC]
        for b in range(BC):
            out[c * BC + b] = outT[:, b]
    return out


# revision 16
# speedup vs baseline: 1.0132x; 1.0030x over previous
"""Trainium2 Bass kernel for DeepMinAttLSTM (4x minLSTM + MHSA + last-step FC).

Strategy:
  - Data-parallel over batch: 16 batches -> 8 cores x 2 batches.
  - Activations feature-major: X^T [H=1024 (8 tiles of 128), B*S=2048] bf16,
    plus an fp8 e4m3 copy (x*16) for the gate matmuls.
  - Per layer: f/i gate matmuls in fp8 with DoubleRow (2x PE throughput,
    W*256 / x*16 scaling, descale 1/4096 folded into the activation),
    h~ matmul in bf16. Gate math: f,i sigmoids on ACT, d=f+i on GpSimd
    (offload), r=1/d via DVE reciprocal_approx_fast, fp=f*r, and the
    input-gate branch uses ip = 1-fp exactly: add = h~' - fp*h~'.
  - Recurrence via DVE tensor_tensor_scan along time (fp32 state).
  - Attention collapses to the last query position; the full K matmul is
    eliminated via q~_bj = Wk_j^T q_bj so scores_bjk = q~_bj . h4_bk
    (K bias shifts all logits equally -> cancels in softmax). Scores, V,
    and the o-accumulation all run in fp8 DoubleRow.
  - Expected rel err ~1.2e-2 (fp8 gates dominate; threshold 2e-2).
"""

import math

import numpy as np
import ml_dtypes

BF16 = ml_dtypes.bfloat16
F8E4 = ml_dtypes.float8_e4m3

P = 128
H = 1024
S = 1024
B = 16
NCORES = 8
BC = B // NCORES          # batches per core
BS = BC * S               # 2048 free columns per core
KO = H // P               # 8 feature partition-tiles
NH = 8
DH = H // NH              # 128
O = 256
L = 4
QSCALE = 1.0 / math.sqrt(DH)
XS = 16.0                 # fp8 scale for activations
WS = 256.0                # fp8 scale for gate weights
ES = 64.0                 # fp8 scale for attention small tensors
GSC = 1.0 / (XS * WS)     # psum descale for fp8 gate matmuls

_CACHE = {}


def _build_nc():
    import concourse.mybir as mybir
    import concourse.tile as tile
    from concourse import bacc

    DT = mybir.dt.bfloat16
    F8 = mybir.dt.float8e4
    F32 = mybir.dt.float32
    AFT = mybir.ActivationFunctionType
    OP = mybir.AluOpType
    DR = mybir.MatmulPerfMode.DoubleRow

    nc = bacc.Bacc("TRN2", target_bir_lowering=False, debug=False,
                   num_devices=NCORES)

    xT8 = nc.dram_tensor("xT8", [P, KO * BS], F8, kind="ExternalInput").ap()
    xTb = nc.dram_tensor("xTb", [P, KO * BS], DT, kind="ExternalInput").ap()
    gw8 = nc.dram_tensor("gw8", [2 * L * P, KO * H], F8,
                         kind="ExternalInput").ap()
    gwh = nc.dram_tensor("gwh", [L * P, KO * H], DT, kind="ExternalInput").ap()
    gnb = nc.dram_tensor("gnb", [P, 2 * L * KO], F32,
                         kind="ExternalInput").ap()
    ghb = nc.dram_tensor("ghb", [P, L * KO], F32, kind="ExternalInput").ap()
    wq = nc.dram_tensor("wq", [P, KO * H], DT, kind="ExternalInput").ap()
    wk = nc.dram_tensor("wk", [P, NH * H], DT, kind="ExternalInput").ap()
    wv = nc.dram_tensor("wv", [P, KO * H], F8, kind="ExternalInput").ap()
    qb = nc.dram_tensor("qb", [P, KO], F32, kind="ExternalInput").ap()
    vb = nc.dram_tensor("vb", [P, NH], F32, kind="ExternalInput").ap()
    ow = nc.dram_tensor("ow", [P, KO * H], DT, kind="ExternalInput").ap()
    ob = nc.dram_tensor("ob", [P, KO], F32, kind="ExternalInput").ap()
    fcw = nc.dram_tensor("fcw", [P, KO * O], DT, kind="ExternalInput").ap()
    fcb = nc.dram_tensor("fcb", [P, O // P], F32, kind="ExternalInput").ap()
    idm = nc.dram_tensor("idm", [P, P], F32, kind="ExternalInput").ap()
    outT = nc.dram_tensor("outT", [O, BC], F32, kind="ExternalOutput").ap()

    with tile.TileContext(nc) as tc:
        with (
            tc.tile_pool(name="constp", bufs=1) as constp,
            tc.tile_pool(name="hbuf", bufs=2) as hp,
        ):
            gnb_sb = constp.tile([P, 2 * L * KO], F32)
            ghb_sb = constp.tile([P, L * KO], F32)
            wv_sb = constp.tile([P, KO * H], F8)
            nc.gpsimd.dma_start(gnb_sb[:], gnb[:])
            nc.gpsimd.dma_start(ghb_sb[:], ghb[:])

            x8p = tc.tile_pool(name="x8p", bufs=2)
            x8pool = x8p.__enter__()

            X = hp.tile([P, KO * BS], DT, tag="hbuf", name="xT_sb")
            X8 = x8pool.tile([P, KO * BS], F8, tag="x8", name="x8_0")
            xT8_v = xT8.rearrange("p (k m) -> p k m", k=KO)
            xTb_v = xTb.rearrange("p (k m) -> p k m", k=KO)
            X_v = X.rearrange("p (k m) -> p k m", k=KO)
            X8_v = X8.rearrange("p (k m) -> p k m", k=KO)
            for ch in range(4):
                m0 = ch * 512
                nc.sync.dma_start(X8_v[:, :, m0:m0 + 512],
                                  xT8_v[:, :, m0:m0 + 512])
                nc.scalar.dma_start(X_v[:, :, m0:m0 + 512],
                                    xTb_v[:, :, m0:m0 + 512])

            # ---------------- minLSTM layers ----------------
            with (
                tc.tile_pool(name="gw8p", bufs=2) as gw8p,
                tc.tile_pool(name="gwhp", bufs=2) as gwhp,
                tc.tile_pool(name="fpp", bufs=3) as fpp,
                tc.tile_pool(name="addp", bufs=3) as addp,
                tc.tile_pool(name="tmpp", bufs=3) as tmpp,
                tc.tile_pool(name="psA", bufs=6, space="PSUM") as psA,
            ):
                for l in range(L):
                    if l == 1:
                        nc.sync.dma_start(wv_sb[:], wv[:])
                    gwf = gw8p.tile([P, KO * H], F8, tag="gwf",
                                    name=f"gwf_{l}")
                    gwi = gw8p.tile([P, KO * H], F8, tag="gwi",
                                    name=f"gwi_{l}")
                    gwhb = gwhp.tile([P, KO * H], DT, tag="gwh",
                                     name=f"gwh_{l}")
                    gwf_v = gwf.rearrange("p (k h) -> p k h", k=KO)
                    gwi_v = gwi.rearrange("p (k h) -> p k h", k=KO)
                    gw8_v = gw8.rearrange("(g p) (k h) -> g p k h", p=P, k=KO)
                    # chunked loads so layer-0 matmuls start early
                    for kp in range(4):
                        nc.gpsimd.dma_start(
                            gwf_v[:, 2 * kp:2 * kp + 2, :],
                            gw8_v[2 * l, :, 2 * kp:2 * kp + 2, :])
                    for kp in range(4):
                        nc.gpsimd.dma_start(
                            gwi_v[:, 2 * kp:2 * kp + 2, :],
                            gw8_v[2 * l + 1, :, 2 * kp:2 * kp + 2, :])
                    for hf in range(2):
                        nc.gpsimd.dma_start(
                            gwhb[:, hf * 4 * H:(hf + 1) * 4 * H],
                            gwh[l * P:(l + 1) * P,
                                hf * 4 * H:(hf + 1) * 4 * H])

                    h_out = hp.tile([P, KO * BS], DT, tag="hbuf",
                                    name=f"h_{l}")
                    X8n = x8pool.tile([P, KO * BS], F8, tag="x8",
                                      name=f"x8_{l + 1}")
                    X8c_v = X8.rearrange("p (k m) -> p k m", k=KO)
                    for no in range(KO):
                        for ch in range(4):
                            m0 = ch * 512
                            b, half = ch // 2, ch % 2
                            base = no * BS + b * S
                            s0 = base + half * 512
                            psF = psA.tile([P, 512], F32, tag="ps", name="psF")
                            psI = psA.tile([P, 512], F32, tag="ps", name="psI")
                            psH = psA.tile([P, 512], F32, tag="ps", name="psH")
                            for kp in range(4):
                                nc.tensor.matmul(
                                    psF[:],
                                    gwf_v[:, 2 * kp:2 * kp + 2,
                                          no * P:(no + 1) * P],
                                    X8c_v[:, 2 * kp:2 * kp + 2, m0:m0 + 512],
                                    start=(kp == 0), stop=(kp == 3),
                                    perf_mode=DR)
                            for kp in range(4):
                                nc.tensor.matmul(
                                    psI[:],
                                    gwi_v[:, 2 * kp:2 * kp + 2,
                                          no * P:(no + 1) * P],
                                    X8c_v[:, 2 * kp:2 * kp + 2, m0:m0 + 512],
                                    start=(kp == 0), stop=(kp == 3),
                                    perf_mode=DR)
                            for ko in range(KO):
                                nc.tensor.matmul(
                                    psH[:],
                                    gwhb[:, ko * H + no * P:
                                         ko * H + (no + 1) * P],
                                    X[:, ko * BS + m0: ko * BS + m0 + 512],
                                    start=(ko == 0), stop=(ko == KO - 1))
                            bF = gnb_sb[:, (l * 2 + 0) * KO + no:
                                        (l * 2 + 0) * KO + no + 1]
                            bI = gnb_sb[:, (l * 2 + 1) * KO + no:
                                        (l * 2 + 1) * KO + no + 1]
                            bH = ghb_sb[:, l * KO + no: l * KO + no + 1]
                            f_t = tmpp.tile([P, 512], DT, tag="f", name="f_t")
                            i_t = tmpp.tile([P, 512], DT, tag="i", name="i_t")
                            d_t = tmpp.tile([P, 512], F32, tag="d", name="d_t")
                            r_t = tmpp.tile([P, 512], F32, tag="r", name="r_t")
                            ip_t = tmpp.tile([P, 512], DT, tag="ip",
                                             name="ip_t")
                            fp_t = fpp.tile([P, 512], DT, tag="fp",
                                            name="fp_t")
                            add_t = addp.tile([P, 512], DT, tag="add",
                                              name="add_t")
                            nc.scalar.activation(f_t[:], psF[:], AFT.Sigmoid,
                                                 bias=bF, scale=GSC)
                            nc.scalar.activation(i_t[:], psI[:], AFT.Sigmoid,
                                                 bias=bI, scale=GSC)
                            # d = f+i on GpSimd (DVE offload); fp = f/d;
                            # ip = 1-fp exactly; add = (zh+bh)*ip straight
                            # from PSUM (no h~ copy needed)
                            nc.gpsimd.tensor_add(d_t[:], f_t[:], i_t[:])
                            nc.vector.reciprocal_approx_fast(r_t[:], d_t[:])
                            nc.vector.tensor_mul(fp_t[:], f_t[:], r_t[:])
                            nc.vector.tensor_scalar(ip_t[:], fp_t[:], -1.0,
                                                    1.0, op0=OP.mult,
                                                    op1=OP.add)
                            nc.vector.scalar_tensor_tensor(
                                add_t[:], psH[:], bH, ip_t[:],
                                op0=OP.add, op1=OP.mult)
                            if half == 0:
                                nc.vector.tensor_tensor_scan(
                                    h_out[:, s0:s0 + 512],
                                    fp_t[:], add_t[:],
                                    initial=0.0, op0=OP.mult, op1=OP.add)
                            else:
                                nc.vector.tensor_tensor_scan(
                                    h_out[:, s0:s0 + 512],
                                    fp_t[:], add_t[:],
                                    initial=h_out[:, s0 - 1:s0],
                                    op0=OP.mult, op1=OP.add)
                            nc.scalar.activation(
                                X8n[:, s0:s0 + 512], h_out[:, s0:s0 + 512],
                                AFT.Copy, scale=XS)
                    X = h_out
                    X8 = X8n

            h4 = X
            h4_8 = X8
            h48_v = h4_8.rearrange("p (k m) -> p k m", k=KO)

            # ---------------- attention (last query position only) ----------
            with (
                tc.tile_pool(name="awp", bufs=1) as awp,
                tc.tile_pool(name="vp", bufs=1) as vp,
                tc.tile_pool(name="smallp", bufs=1) as smallp,
            ):
                wq_sb = awp.tile([P, KO * H], DT)
                nc.sync.dma_start(wq_sb[:], wq[:])
                wk_sb = awp.tile([P, NH * H], DT)
                nc.sync.dma_start(wk_sb[:], wk[:])
                wv_v = wv_sb.rearrange("p (k h) -> p k h", k=KO)
                ow_sb = awp.tile([P, KO * H], DT)
                nc.sync.dma_start(ow_sb[:], ow[:])
                fcw_sb = awp.tile([P, KO * O], DT)
                nc.sync.dma_start(fcw_sb[:], fcw[:])
                qb_sb = constp.tile([P, KO], F32)
                nc.gpsimd.dma_start(qb_sb[:], qb[:])
                vb_sb = constp.tile([P, NH], F32)
                nc.gpsimd.dma_start(vb_sb[:], vb[:])
                ob_sb = constp.tile([P, KO], F32)
                nc.gpsimd.dma_start(ob_sb[:], ob[:])
                fcb_sb = constp.tile([P, O // P], F32)
                nc.gpsimd.dma_start(fcb_sb[:], fcb[:])
                idm_sb = constp.tile([P, P], F32)
                nc.gpsimd.dma_start(idm_sb[:], idm[:])

                V_sb = vp.tile([P, BC * KO * H], F8, name="V_sb")
                V_v = V_sb.rearrange("p (b k h) -> p b k h", b=BC, k=KO)
                lastq = smallp.tile([P, KO * BC], DT)     # col = ko*2 + b
                q_sb = smallp.tile([P, KO * BC], DT)      # col = nt*2 + b
                qt_fm = smallp.tile([P, KO * NH * BC], F8)  # col=(dt*8+j)*2+b
                e_sb = smallp.tile([NH, BC * S], DT)      # col = b*S + s
                en_sb = smallp.tile([NH, BC * S], F32)
                acc = smallp.tile([NH, 2 * BC], F32)      # col = b*2 + ch
                den = smallp.tile([NH, BC], F32)
                rden = smallp.tile([NH, BC], F32)
                eT_sb = smallp.tile([P, BC * KO * NH], F8)  # col=(b*8+kt)*8+j
                eT_v = eT_sb.rearrange("p (b k j) -> p b k j", b=BC, k=KO)
                O_last = smallp.tile([P, NH * BC], DT)    # col = j*2 + b
                out_last = smallp.tile([P, KO * BC], DT)
                res_sb = smallp.tile([P, 2 * (O // P)], F32)

                # h4 columns at the last timestep: one strided copy
                h4_l = h4.rearrange("p (k b s) -> p k b s", k=KO, b=BC)
                lq_v = lastq.rearrange("p (k b) -> p k b", k=KO)
                nc.vector.tensor_copy(lq_v[:, :, :],
                                      h4_l[:, :, :, S - 1:S])

                with (
                    tc.tile_pool(name="psS", bufs=2, space="PSUM") as psS,
                    tc.tile_pool(name="psT", bufs=2, space="PSUM") as psT,
                    tc.tile_pool(name="psO", bufs=1, space="PSUM") as psO,
                    tc.tile_pool(name="psV", bufs=3, space="PSUM") as psV,
                ):
                    # V position-major [BS, H] via fp8 DoubleRow, stored fp8
                    for st in range(BC * KO):
                        b, si = st // KO, st % KO
                        m0 = b * S + si * P
                        for dch in range(2):
                            d0 = dch * 512
                            ps = psV.tile([P, 512], F32, tag="v", name="psv")
                            for kp in range(4):
                                nc.tensor.matmul(
                                    ps[:],
                                    h48_v[:, 2 * kp:2 * kp + 2, m0:m0 + P],
                                    wv_v[:, 2 * kp:2 * kp + 2, d0:d0 + 512],
                                    start=(kp == 0), stop=(kp == 3),
                                    perf_mode=DR)
                            nc.scalar.activation(
                                V_sb[:, st * H + d0: st * H + d0 + 512],
                                ps[:], AFT.Copy, scale=GSC * ES)

                    # q at the last position (scaled by QSCALE via bias+scale)
                    for nt in range(KO):
                        ps = psS.tile([P, BC], F32, tag="s", name="psq")
                        for ko in range(KO):
                            nc.tensor.matmul(
                                ps[:],
                                wq_sb[:, ko * H + nt * P:
                                      ko * H + (nt + 1) * P],
                                lastq[:, ko * BC:(ko + 1) * BC],
                                start=(ko == 0), stop=(ko == KO - 1))
                        nc.scalar.activation(
                            q_sb[:, nt * BC:(nt + 1) * BC], ps[:],
                            AFT.Identity, bias=qb_sb[:, nt:nt + 1],
                            scale=QSCALE)
                    # q~ feature-major fp8: q~[dt-tile, (j, b)] = Wk_j^T q_bj
                    for dt in range(KO):
                        ps = psS.tile([P, NH * BC], F32, tag="s", name="psqt")
                        for j in range(NH):
                            nc.tensor.matmul(
                                ps[:, j * BC:(j + 1) * BC],
                                wk_sb[:, j * H + dt * P: j * H + (dt + 1) * P],
                                q_sb[:, j * BC:(j + 1) * BC],
                                start=True, stop=True)
                        nc.scalar.activation(
                            qt_fm[:, dt * NH * BC:(dt + 1) * NH * BC], ps[:],
                            AFT.Copy, scale=ES)
                    # scores [8 heads, 512 pos] per (b, ch) via fp8 DR + exp
                    qt_v = qt_fm.rearrange("p (k j b) -> p k j b", k=KO, j=NH)
                    for b in range(BC):
                        for ch in range(2):
                            m0 = b * S + ch * 512
                            ps = psS.tile([NH, 512], F32, tag="s",
                                          name="pssc")
                            for kp in range(4):
                                nc.tensor.matmul(
                                    ps[:],
                                    qt_v[:, 2 * kp:2 * kp + 2, :, b:b + 1],
                                    h48_v[:, 2 * kp:2 * kp + 2, m0:m0 + 512],
                                    start=(kp == 0), stop=(kp == 3),
                                    perf_mode=DR)
                            nc.scalar.activation(
                                e_sb[:, m0:m0 + 512], ps[:], AFT.Exp,
                                scale=1.0 / (ES * XS),
                                accum_out=acc[:, b * 2 + ch:b * 2 + ch + 1])
                    # denominators and normalized weights (fp8, *ES)
                    acc_v = acc.rearrange("p (b c) -> p b c", b=BC)
                    nc.vector.tensor_add(den[:, :], acc_v[:, :, 0:1],
                                         acc_v[:, :, 1:2])
                    nc.vector.reciprocal_approx_fast(rden[:, :], den[:, :])
                    for b in range(BC):
                        nc.scalar.activation(
                            en_sb[:, b * S:(b + 1) * S],
                            e_sb[:, b * S:(b + 1) * S],
                            AFT.Copy, scale=rden[:, b:b + 1])
                    # transpose normalized weights to [pos, head], cast fp8
                    for b in range(BC):
                        for kt in range(KO):
                            pst = psT.tile([P, NH], F32, tag="t", name="pst")
                            nc.tensor.transpose(
                                pst[:],
                                en_sb[:, b * S + kt * P: b * S + (kt + 1) * P],
                                idm_sb[0:NH, 0:NH])
                            nc.scalar.activation(
                                eT_sb[:, (b * KO + kt) * NH:
                                      (b * KO + kt + 1) * NH],
                                pst[:], AFT.Copy, scale=ES)

                    # o at last position via fp8 DoubleRow over position tiles
                    ps_ob = psO.tile([P, NH * BC], F32, tag="o", name="ps_ob")
                    for b in range(BC):
                        for j in range(NH):
                            c = j * BC + b
                            for kp in range(4):
                                nc.tensor.matmul(
                                    ps_ob[:, c:c + 1],
                                    V_v[:, b, 2 * kp:2 * kp + 2,
                                        j * P:(j + 1) * P],
                                    eT_v[:, b, 2 * kp:2 * kp + 2, j:j + 1],
                                    start=(kp == 0), stop=(kp == 3),
                                    perf_mode=DR)
                    for j in range(NH):
                        nc.scalar.activation(
                            O_last[:, j * BC:(j + 1) * BC],
                            ps_ob[:, j * BC:(j + 1) * BC],
                            AFT.Identity, bias=vb_sb[:, j:j + 1],
                            scale=1.0 / (ES * ES))
                    # out projection + residual
                    for no in range(KO):
                        ps = psS.tile([P, BC], F32, tag="s", name="psp")
                        for ko in range(KO):
                            nc.tensor.matmul(
                                ps[:],
                                ow_sb[:, ko * H + no * P: ko * H + (no + 1) * P],
                                O_last[:, ko * BC:(ko + 1) * BC],
                                start=(ko == 0), stop=(ko == KO - 1))
                        nc.vector.scalar_tensor_tensor(
                            out_last[:, no * BC:(no + 1) * BC],
                            ps[:], ob_sb[:, no:no + 1],
                            lastq[:, no * BC:(no + 1) * BC],
                            op0=OP.add, op1=OP.add)
                    # final fc
                    for ot in range(O // P):
                        ps = psS.tile([P, BC], F32, tag="s", name="psf")
                        for ko in range(KO):
                            nc.tensor.matmul(
                                ps[:],
                                fcw_sb[:, ko * O + ot * P: ko * O + (ot + 1) * P],
                                out_last[:, ko * BC:(ko + 1) * BC],
                                start=(ko == 0), stop=(ko == KO - 1))
                        nc.scalar.activation(
                            res_sb[:, ot * BC:(ot + 1) * BC], ps[:],
                            AFT.Identity, bias=fcb_sb[:, ot:ot + 1])
                        nc.sync.dma_start(
                            outT[ot * P:(ot + 1) * P, :],
                            res_sb[:, ot * BC:(ot + 1) * BC])

            x8p.__exit__(None, None, None)

    nc.compile()
    return nc


def _feature_major(w_t):
    """[H_in, N] (already transposed weight) -> device layout [128, KO*N]."""
    hin, n = w_t.shape
    ko = hin // P
    return np.ascontiguousarray(
        w_t.reshape(ko, P, n).transpose(1, 0, 2).reshape(P, ko * n))


def _prep_inputs(x, Wf, bf, Wi, bi, Wh, bh, in_proj_w, in_proj_b, out_w,
                 out_b, fc_w, fc_b):
    gw8s, gwhs, gnbs, ghbs = [], [], [], []
    for l in range(L):
        for W, bias in ((Wf[l], bf[l]), (Wi[l], bi[l])):
            fm = _feature_major(W.T.astype(np.float32) * WS)
            gw8s.append(fm.astype(F8E4))
            gnbs.append(bias.reshape(KO, P).T.astype(np.float32))
        gwhs.append(_feature_major(Wh[l].T.astype(np.float32)).astype(BF16))
        ghbs.append(bh[l].reshape(KO, P).T.astype(np.float32))
    gw8 = np.concatenate(gw8s, axis=0)                   # [2L*128, KO*H]
    gwh = np.concatenate(gwhs, axis=0)                   # [L*128, KO*H]
    gnb = np.ascontiguousarray(np.concatenate(gnbs, axis=1))
    ghb = np.ascontiguousarray(np.concatenate(ghbs, axis=1))
    ipw = in_proj_w.astype(np.float32)
    wq_ = _feature_major(ipw[:H].T).astype(BF16)
    wk_ = np.ascontiguousarray(
        ipw[H:2 * H].reshape(NH, P, H).transpose(1, 0, 2).reshape(P, NH * H)
    ).astype(BF16)
    wv_ = (_feature_major(ipw[2 * H:].T) * WS).astype(F8E4)
    qb_ = np.ascontiguousarray(
        (in_proj_b[:H] * QSCALE).reshape(KO, P).T.astype(np.float32))
    vb_ = np.ascontiguousarray(
        in_proj_b[2 * H:].reshape(NH, P).T.astype(np.float32))
    ow_ = _feature_major(out_w.T.astype(np.float32)).astype(BF16)
    ob_ = np.ascontiguousarray(out_b.reshape(KO, P).T.astype(np.float32))
    fcw_ = _feature_major(fc_w.T.astype(np.float32)).astype(BF16)
    fcb_ = np.ascontiguousarray(
        fc_b.reshape(O // P, P).T.astype(np.float32))
    idm_ = np.eye(P, dtype=np.float32)
    shared = dict(gw8=gw8, gwh=gwh, gnb=gnb, ghb=ghb, wq=wq_, wk=wk_, wv=wv_,
                  qb=qb_, vb=vb_, ow=ow_, ob=ob_, fcw=fcw_, fcb=fcb_,
                  idm=idm_)
    in_maps = []
    for c in range(NCORES):
        shard = x[c * BC:(c + 1) * BC]                   # [BC, S, H]
        xt = shard.transpose(2, 0, 1).reshape(H, BS)     # [H, BS]
        xt = _feature_major(xt.astype(np.float32))       # [128, KO*BS]
        in_maps.append(dict(shared, xT8=(xt * XS).astype(F8E4),
                            xTb=xt.astype(BF16)))
    return in_maps


def kernel(x, Wf, bf, Wi, bi, Wh, bh, in_proj_w, in_proj_b, out_w, out_b,
           fc_w, fc_b):
    from concourse.bass_utils import run_bass_kernel_spmd

    x, Wf, bf, Wi, bi, Wh, bh = (np.asarray(t) for t in
                                 (x, Wf, bf, Wi, bi, Wh, bh))
    in_proj_w, in_proj_b, out_w, out_b, fc_w, fc_b = (
        np.asarray(t) for t in (in_proj_w, in_proj_b, out_w, out_b,
                                fc_w, fc_b))
    if "nc" not in _CACHE:
        _CACHE["nc"] = _build_nc()
    nc = _CACHE["nc"]
    in_maps = _prep_inputs(x, Wf, bf, Wi, bi, Wh, bh, in_proj_w, in_proj_b,
                           out_w, out_b, fc_w, fc_b)
    res = run_bass_kernel_spmd(nc, in_maps, core_ids=list(range(NCORES)))
    _CACHE["last_results"] = res
    out = np.empty((B, O), np.float32)
    for c in range(NCORES):
        outT = res.results[c]["outT"]                    # [O, BC]
        for b in range(BC):
            out[c * BC + b] = outT[:, b]
    return out


# revision 17
# speedup vs baseline: 1.0192x; 1.0059x over previous
"""Trainium2 Bass kernel for DeepMinAttLSTM (4x minLSTM + MHSA + last-step FC).

Strategy:
  - Data-parallel over batch: 16 batches -> 8 cores x 2 batches.
  - Activations feature-major: X^T [H=1024 (8 tiles of 128), B*S=2048] bf16,
    plus an fp8 e4m3 copy (x*16) for the gate matmuls.
  - Per layer: f/i gate matmuls in fp8 with DoubleRow (2x PE throughput,
    W*256 / x*16 scaling, descale 1/4096 folded into the activation),
    h~ matmul in bf16. Gate math: f,i sigmoids on ACT, d=f+i on GpSimd
    (offload), r=1/d via DVE reciprocal_approx_fast, fp=f*r, and the
    input-gate branch uses ip = 1-fp exactly: add = h~' - fp*h~'.
  - Recurrence via DVE tensor_tensor_scan along time (fp32 state).
  - Attention collapses to the last query position; the full K matmul is
    eliminated via q~_bj = Wk_j^T q_bj so scores_bjk = q~_bj . h4_bk
    (K bias shifts all logits equally -> cancels in softmax). Scores, V,
    and the o-accumulation all run in fp8 DoubleRow.
  - Expected rel err ~1.2e-2 (fp8 gates dominate; threshold 2e-2).
"""

import math

import numpy as np
import ml_dtypes

BF16 = ml_dtypes.bfloat16
F8E4 = ml_dtypes.float8_e4m3

P = 128
H = 1024
S = 1024
B = 16
NCORES = 8
BC = B // NCORES          # batches per core
BS = BC * S               # 2048 free columns per core
KO = H // P               # 8 feature partition-tiles
NH = 8
DH = H // NH              # 128
O = 256
L = 4
QSCALE = 1.0 / math.sqrt(DH)
XS = 16.0                 # fp8 scale for activations
WS = 256.0                # fp8 scale for gate weights
ES = 64.0                 # fp8 scale for attention small tensors
GSC = 1.0 / (XS * WS)     # psum descale for fp8 gate matmuls

_CACHE = {}


def _build_nc():
    import concourse.mybir as mybir
    import concourse.tile as tile
    from concourse import bacc

    DT = mybir.dt.bfloat16
    F8 = mybir.dt.float8e4
    F32 = mybir.dt.float32
    AFT = mybir.ActivationFunctionType
    OP = mybir.AluOpType
    DR = mybir.MatmulPerfMode.DoubleRow

    nc = bacc.Bacc("TRN2", target_bir_lowering=False, debug=False,
                   num_devices=NCORES)

    xT8 = nc.dram_tensor("xT8", [P, KO * BS], F8, kind="ExternalInput").ap()
    xTb = nc.dram_tensor("xTb", [P, KO * BS], DT, kind="ExternalInput").ap()
    gw8 = nc.dram_tensor("gw8", [2 * L * P, KO * H], F8,
                         kind="ExternalInput").ap()
    gwh = nc.dram_tensor("gwh", [L * P, KO * H], DT, kind="ExternalInput").ap()
    gnb = nc.dram_tensor("gnb", [P, 2 * L * KO], F32,
                         kind="ExternalInput").ap()
    ghb = nc.dram_tensor("ghb", [P, L * KO], F32, kind="ExternalInput").ap()
    wq = nc.dram_tensor("wq", [P, KO * H], DT, kind="ExternalInput").ap()
    wk = nc.dram_tensor("wk", [P, NH * H], DT, kind="ExternalInput").ap()
    wv = nc.dram_tensor("wv", [P, KO * H], F8, kind="ExternalInput").ap()
    qb = nc.dram_tensor("qb", [P, KO], F32, kind="ExternalInput").ap()
    vb = nc.dram_tensor("vb", [P, NH], F32, kind="ExternalInput").ap()
    ow = nc.dram_tensor("ow", [P, KO * H], DT, kind="ExternalInput").ap()
    ob = nc.dram_tensor("ob", [P, KO], F32, kind="ExternalInput").ap()
    fcw = nc.dram_tensor("fcw", [P, KO * O], DT, kind="ExternalInput").ap()
    fcb = nc.dram_tensor("fcb", [P, O // P], F32, kind="ExternalInput").ap()
    idm = nc.dram_tensor("idm", [P, P], F32, kind="ExternalInput").ap()
    outT = nc.dram_tensor("outT", [O, BC], F32, kind="ExternalOutput").ap()

    with tile.TileContext(nc) as tc:
        with (
            tc.tile_pool(name="constp", bufs=1) as constp,
            tc.tile_pool(name="hbuf", bufs=2) as hp,
        ):
            gnb_sb = constp.tile([P, 2 * L * KO], F32)
            ghb_sb = constp.tile([P, L * KO], F32)
            wv_sb = constp.tile([P, KO * H], F8)
            nc.gpsimd.dma_start(gnb_sb[:], gnb[:])
            nc.gpsimd.dma_start(ghb_sb[:], ghb[:])

            x8p = tc.tile_pool(name="x8p", bufs=2)
            x8pool = x8p.__enter__()

            X = hp.tile([P, KO * BS], DT, tag="hbuf", name="xT_sb")
            X8 = x8pool.tile([P, KO * BS], F8, tag="x8", name="x8_0")
            xT8_v = xT8.rearrange("p (k m) -> p k m", k=KO)
            xTb_v = xTb.rearrange("p (k m) -> p k m", k=KO)
            X_v = X.rearrange("p (k m) -> p k m", k=KO)
            X8_v = X8.rearrange("p (k m) -> p k m", k=KO)
            for ch in range(4):
                m0 = ch * 512
                nc.sync.dma_start(X8_v[:, :, m0:m0 + 512],
                                  xT8_v[:, :, m0:m0 + 512])
                nc.scalar.dma_start(X_v[:, :, m0:m0 + 512],
                                    xTb_v[:, :, m0:m0 + 512])

            # ---------------- minLSTM layers ----------------
            with (
                tc.tile_pool(name="gw8p", bufs=2) as gw8p,
                tc.tile_pool(name="gwhp", bufs=2) as gwhp,
                tc.tile_pool(name="fpp", bufs=3) as fpp,
                tc.tile_pool(name="addp", bufs=3) as addp,
                tc.tile_pool(name="tmpp", bufs=3) as tmpp,
                tc.tile_pool(name="psA", bufs=6, space="PSUM") as psA,
            ):
                for l in range(L):
                    if l == 1:
                        nc.sync.dma_start(wv_sb[:], wv[:])
                    gwf = gw8p.tile([P, KO * H], F8, tag="gwf",
                                    name=f"gwf_{l}")
                    gwi = gw8p.tile([P, KO * H], F8, tag="gwi",
                                    name=f"gwi_{l}")
                    gwhb = gwhp.tile([P, KO * H], DT, tag="gwh",
                                     name=f"gwh_{l}")
                    gwf_v = gwf.rearrange("p (k h) -> p k h", k=KO)
                    gwi_v = gwi.rearrange("p (k h) -> p k h", k=KO)
                    gw8_v = gw8.rearrange("(g p) (k h) -> g p k h", p=P, k=KO)
                    # chunked loads so layer-0 matmuls start early
                    for kp in range(4):
                        nc.gpsimd.dma_start(
                            gwf_v[:, 2 * kp:2 * kp + 2, :],
                            gw8_v[2 * l, :, 2 * kp:2 * kp + 2, :])
                    for kp in range(4):
                        nc.gpsimd.dma_start(
                            gwi_v[:, 2 * kp:2 * kp + 2, :],
                            gw8_v[2 * l + 1, :, 2 * kp:2 * kp + 2, :])
                    for hf in range(2):
                        nc.gpsimd.dma_start(
                            gwhb[:, hf * 4 * H:(hf + 1) * 4 * H],
                            gwh[l * P:(l + 1) * P,
                                hf * 4 * H:(hf + 1) * 4 * H])

                    h_out = hp.tile([P, KO * BS], DT, tag="hbuf",
                                    name=f"h_{l}")
                    X8n = x8pool.tile([P, KO * BS], F8, tag="x8",
                                      name=f"x8_{l + 1}")
                    X8c_v = X8.rearrange("p (k m) -> p k m", k=KO)
                    for no in range(KO):
                        for ch in range(4):
                            m0 = ch * 512
                            b, half = ch // 2, ch % 2
                            base = no * BS + b * S
                            s0 = base + half * 512
                            psF = psA.tile([P, 512], F32, tag="ps", name="psF")
                            psI = psA.tile([P, 512], F32, tag="ps", name="psI")
                            psH = psA.tile([P, 512], F32, tag="ps", name="psH")
                            for kp in range(4):
                                nc.tensor.matmul(
                                    psF[:],
                                    gwf_v[:, 2 * kp:2 * kp + 2,
                                          no * P:(no + 1) * P],
                                    X8c_v[:, 2 * kp:2 * kp + 2, m0:m0 + 512],
                                    start=(kp == 0), stop=(kp == 3),
                                    perf_mode=DR)
                            for kp in range(4):
                                nc.tensor.matmul(
                                    psI[:],
                                    gwi_v[:, 2 * kp:2 * kp + 2,
                                          no * P:(no + 1) * P],
                                    X8c_v[:, 2 * kp:2 * kp + 2, m0:m0 + 512],
                                    start=(kp == 0), stop=(kp == 3),
                                    perf_mode=DR)
                            for ko in range(KO):
                                nc.tensor.matmul(
                                    psH[:],
                                    gwhb[:, ko * H + no * P:
                                         ko * H + (no + 1) * P],
                                    X[:, ko * BS + m0: ko * BS + m0 + 512],
                                    start=(ko == 0), stop=(ko == KO - 1))
                            bF = gnb_sb[:, (l * 2 + 0) * KO + no:
                                        (l * 2 + 0) * KO + no + 1]
                            bI = gnb_sb[:, (l * 2 + 1) * KO + no:
                                        (l * 2 + 1) * KO + no + 1]
                            bH = ghb_sb[:, l * KO + no: l * KO + no + 1]
                            f_t = tmpp.tile([P, 512], DT, tag="f", name="f_t")
                            i_t = tmpp.tile([P, 512], DT, tag="i", name="i_t")
                            d_t = tmpp.tile([P, 512], F32, tag="d", name="d_t")
                            r_t = tmpp.tile([P, 512], F32, tag="r", name="r_t")
                            ip_t = tmpp.tile([P, 512], DT, tag="ip",
                                             name="ip_t")
                            fp_t = fpp.tile([P, 512], DT, tag="fp",
                                            name="fp_t")
                            add_t = addp.tile([P, 512], DT, tag="add",
                                              name="add_t")
                            nc.scalar.activation(f_t[:], psF[:], AFT.Sigmoid,
                                                 bias=bF, scale=GSC)
                            nc.scalar.activation(i_t[:], psI[:], AFT.Sigmoid,
                                                 bias=bI, scale=GSC)
                            # d = f+i on GpSimd (DVE offload); fp = f/d;
                            # ip = 1-fp exactly; add = (zh+bh)*ip straight
                            # from PSUM (no h~ copy needed)
                            nc.gpsimd.tensor_add(d_t[:], f_t[:], i_t[:])
                            nc.vector.reciprocal_approx_fast(r_t[:], d_t[:])
                            nc.vector.tensor_mul(fp_t[:], f_t[:], r_t[:])
                            nc.vector.tensor_scalar(ip_t[:], fp_t[:], -1.0,
                                                    1.0, op0=OP.mult,
                                                    op1=OP.add)
                            nc.vector.scalar_tensor_tensor(
                                add_t[:], psH[:], bH, ip_t[:],
                                op0=OP.add, op1=OP.mult)
                            if half == 0:
                                nc.vector.tensor_tensor_scan(
                                    h_out[:, s0:s0 + 512],
                                    fp_t[:], add_t[:],
                                    initial=0.0, op0=OP.mult, op1=OP.add)
                            else:
                                nc.vector.tensor_tensor_scan(
                                    h_out[:, s0:s0 + 512],
                                    fp_t[:], add_t[:],
                                    initial=h_out[:, s0 - 1:s0],
                                    op0=OP.mult, op1=OP.add)
                            nc.scalar.activation(
                                X8n[:, s0:s0 + 512], h_out[:, s0:s0 + 512],
                                AFT.Copy, scale=XS)
                    X = h_out
                    X8 = X8n

            h4 = X
            h4_8 = X8
            h48_v = h4_8.rearrange("p (k m) -> p k m", k=KO)

            # ---------------- attention (last query position only) ----------
            with (
                tc.tile_pool(name="awp", bufs=1) as awp,
                tc.tile_pool(name="vp", bufs=1) as vp,
                tc.tile_pool(name="smallp", bufs=1) as smallp,
            ):
                wq_sb = awp.tile([P, KO * H], DT)
                nc.sync.dma_start(wq_sb[:], wq[:])
                wk_sb = awp.tile([P, NH * H], DT)
                nc.sync.dma_start(wk_sb[:], wk[:])
                wv_v = wv_sb.rearrange("p (k h) -> p k h", k=KO)
                ow_sb = awp.tile([P, KO * H], DT)
                nc.sync.dma_start(ow_sb[:], ow[:])
                fcw_sb = awp.tile([P, KO * O], DT)
                nc.sync.dma_start(fcw_sb[:], fcw[:])
                qb_sb = constp.tile([P, KO], F32)
                nc.gpsimd.dma_start(qb_sb[:], qb[:])
                vb_sb = constp.tile([P, NH], F32)
                nc.gpsimd.dma_start(vb_sb[:], vb[:])
                ob_sb = constp.tile([P, KO], F32)
                nc.gpsimd.dma_start(ob_sb[:], ob[:])
                fcb_sb = constp.tile([P, O // P], F32)
                nc.gpsimd.dma_start(fcb_sb[:], fcb[:])
                idm_sb = constp.tile([P, P], F32)
                nc.gpsimd.dma_start(idm_sb[:], idm[:])

                V_sb = vp.tile([P, BC * KO * H], F8, name="V_sb")
                V_v = V_sb.rearrange("p (b k h) -> p b k h", b=BC, k=KO)
                lastq = smallp.tile([P, KO * BC], DT)     # col = ko*2 + b
                q_sb = smallp.tile([P, KO * BC], DT)      # col = nt*2 + b
                qt_fm = smallp.tile([P, KO * NH * BC], F8)  # col=(dt*8+j)*2+b
                e_sb = smallp.tile([NH, BC * S], DT)      # col = b*S + s
                en_sb = smallp.tile([NH, BC * S], F32)
                acc = smallp.tile([NH, 2 * BC], F32)      # col = b*2 + ch
                den = smallp.tile([NH, BC], F32)
                rden = smallp.tile([NH, BC], F32)
                eT_sb = smallp.tile([P, BC * KO * NH], F8)  # col=(b*8+kt)*8+j
                eT_v = eT_sb.rearrange("p (b k j) -> p b k j", b=BC, k=KO)
                O_last = smallp.tile([P, NH * BC], DT)    # col = j*2 + b
                out_last = smallp.tile([P, KO * BC], DT)
                res_sb = smallp.tile([P, 2 * (O // P)], F32)

                # h4 columns at the last timestep: one strided copy
                h4_l = h4.rearrange("p (k b s) -> p k b s", k=KO, b=BC)
                lq_v = lastq.rearrange("p (k b) -> p k b", k=KO)
                nc.vector.tensor_copy(lq_v[:, :, :],
                                      h4_l[:, :, :, S - 1:S])

                with (
                    tc.tile_pool(name="psV", bufs=2, space="PSUM") as psV,
                    tc.tile_pool(name="psS", bufs=2, space="PSUM") as psS,
                    tc.tile_pool(name="psT", bufs=2, space="PSUM") as psT,
                    tc.tile_pool(name="psO", bufs=1, space="PSUM") as psO,
                ):
                    # V position-major [BS, H] via fp8 DoubleRow, stored fp8
                    for st in range(BC * KO):
                        b, si = st // KO, st % KO
                        m0 = b * S + si * P
                        for dch in range(2):
                            d0 = dch * 512
                            ps = psV.tile([P, 512], F32, tag="v", name="psv")
                            for kp in range(4):
                                nc.tensor.matmul(
                                    ps[:],
                                    h48_v[:, 2 * kp:2 * kp + 2, m0:m0 + P],
                                    wv_v[:, 2 * kp:2 * kp + 2, d0:d0 + 512],
                                    start=(kp == 0), stop=(kp == 3),
                                    perf_mode=DR)
                            nc.scalar.activation(
                                V_sb[:, st * H + d0: st * H + d0 + 512],
                                ps[:], AFT.Copy, scale=GSC * ES)

                    # q at the last position (scaled by QSCALE via bias+scale)
                    for nt in range(KO):
                        ps = psS.tile([P, BC], F32, tag="s", name="psq")
                        for ko in range(KO):
                            nc.tensor.matmul(
                                ps[:],
                                wq_sb[:, ko * H + nt * P:
                                      ko * H + (nt + 1) * P],
                                lastq[:, ko * BC:(ko + 1) * BC],
                                start=(ko == 0), stop=(ko == KO - 1))
                        nc.scalar.activation(
                            q_sb[:, nt * BC:(nt + 1) * BC], ps[:],
                            AFT.Identity, bias=qb_sb[:, nt:nt + 1],
                            scale=QSCALE)
                    # q~ feature-major fp8: q~[dt-tile, (j, b)] = Wk_j^T q_bj
                    for dt in range(KO):
                        ps = psS.tile([P, NH * BC], F32, tag="s", name="psqt")
                        for j in range(NH):
                            nc.tensor.matmul(
                                ps[:, j * BC:(j + 1) * BC],
                                wk_sb[:, j * H + dt * P: j * H + (dt + 1) * P],
                                q_sb[:, j * BC:(j + 1) * BC],
                                start=True, stop=True)
                        nc.scalar.activation(
                            qt_fm[:, dt * NH * BC:(dt + 1) * NH * BC], ps[:],
                            AFT.Copy, scale=ES)
                    # scores [8 heads, 512 pos] per (b, ch) via fp8 DR + exp
                    qt_v = qt_fm.rearrange("p (k j b) -> p k j b", k=KO, j=NH)
                    for b in range(BC):
                        for ch in range(2):
                            m0 = b * S + ch * 512
                            ps = psS.tile([NH, 512], F32, tag="s",
                                          name="pssc")
                            for kp in range(4):
                                nc.tensor.matmul(
                                    ps[:],
                                    qt_v[:, 2 * kp:2 * kp + 2, :, b:b + 1],
                                    h48_v[:, 2 * kp:2 * kp + 2, m0:m0 + 512],
                                    start=(kp == 0), stop=(kp == 3),
                                    perf_mode=DR)
                            nc.scalar.activation(
                                e_sb[:, m0:m0 + 512], ps[:], AFT.Exp,
                                scale=1.0 / (ES * XS),
                                accum_out=acc[:, b * 2 + ch:b * 2 + ch + 1])
                    # denominators and normalized weights (fp8, *ES)
                    acc_v = acc.rearrange("p (b c) -> p b c", b=BC)
                    nc.vector.tensor_add(den[:, :], acc_v[:, :, 0:1],
                                         acc_v[:, :, 1:2])
                    nc.vector.reciprocal_approx_fast(rden[:, :], den[:, :])
                    for b in range(BC):
                        nc.scalar.activation(
                            en_sb[:, b * S:(b + 1) * S],
                            e_sb[:, b * S:(b + 1) * S],
                            AFT.Copy, scale=rden[:, b:b + 1])
                    # transpose normalized weights to [pos, head], cast fp8
                    for b in range(BC):
                        for kt in range(KO):
                            pst = psT.tile([P, NH], F32, tag="t", name="pst")
                            nc.tensor.transpose(
                                pst[:],
                                en_sb[:, b * S + kt * P: b * S + (kt + 1) * P],
                                idm_sb[0:NH, 0:NH])
                            nc.scalar.activation(
                                eT_sb[:, (b * KO + kt) * NH:
                                      (b * KO + kt + 1) * NH],
                                pst[:], AFT.Copy, scale=ES)

                    # o at last position via fp8 DoubleRow over position tiles
                    ps_ob = psO.tile([P, NH * BC], F32, tag="o", name="ps_ob")
                    for b in range(BC):
                        for j in range(NH):
                            c = j * BC + b
                            for kp in range(4):
                                nc.tensor.matmul(
                                    ps_ob[:, c:c + 1],
                                    V_v[:, b, 2 * kp:2 * kp + 2,
                                        j * P:(j + 1) * P],
                                    eT_v[:, b, 2 * kp:2 * kp + 2, j:j + 1],
                                    start=(kp == 0), stop=(kp == 3),
                                    perf_mode=DR)
                    for j in range(NH):
                        nc.scalar.activation(
                            O_last[:, j * BC:(j + 1) * BC],
                            ps_ob[:, j * BC:(j + 1) * BC],
                            AFT.Identity, bias=vb_sb[:, j:j + 1],
                            scale=1.0 / (ES * ES))
                    # out projection + residual
                    for no in range(KO):
                        ps = psS.tile([P, BC], F32, tag="s", name="psp")
                        for ko in range(KO):
                            nc.tensor.matmul(
                                ps[:],
                                ow_sb[:, ko * H + no * P: ko * H + (no + 1) * P],
                                O_last[:, ko * BC:(ko + 1) * BC],
                                start=(ko == 0), stop=(ko == KO - 1))
                        nc.vector.scalar_tensor_tensor(
                            out_last[:, no * BC:(no + 1) * BC],
                            ps[:], ob_sb[:, no:no + 1],
                            lastq[:, no * BC:(no + 1) * BC],
                            op0=OP.add, op1=OP.add)
                    # final fc
                    for ot in range(O // P):
                        ps = psS.tile([P, BC], F32, tag="s", name="psf")
                        for ko in range(KO):
                            nc.tensor.matmul(
                                ps[:],
                                fcw_sb[:, ko * O + ot * P: ko * O + (ot + 1) * P],
                                out_last[:, ko * BC:(ko + 1) * BC],
                                start=(ko == 0), stop=(ko == KO - 1))
                        nc.scalar.activation(
                            res_sb[:, ot * BC:(ot + 1) * BC], ps[:],
                            AFT.Identity, bias=fcb_sb[:, ot:ot + 1])
                        nc.sync.dma_start(
                            outT[ot * P:(ot + 1) * P, :],
                            res_sb[:, ot * BC:(ot + 1) * BC])

            x8p.__exit__(None, None, None)

    nc.compile()
    return nc


def _feature_major(w_t):
    """[H_in, N] (already transposed weight) -> device layout [128, KO*N]."""
    hin, n = w_t.shape
    ko = hin // P
    return np.ascontiguousarray(
        w_t.reshape(ko, P, n).transpose(1, 0, 2).reshape(P, ko * n))


def _prep_inputs(x, Wf, bf, Wi, bi, Wh, bh, in_proj_w, in_proj_b, out_w,
                 out_b, fc_w, fc_b):
    gw8s, gwhs, gnbs, ghbs = [], [], [], []
    for l in range(L):
        for W, bias in ((Wf[l], bf[l]), (Wi[l], bi[l])):
            fm = _feature_major(W.T.astype(np.float32) * WS)
            gw8s.append(fm.astype(F8E4))
            gnbs.append(bias.reshape(KO, P).T.astype(np.float32))
        gwhs.append(_feature_major(Wh[l].T.astype(np.float32)).astype(BF16))
        ghbs.append(bh[l].reshape(KO, P).T.astype(np.float32))
    gw8 = np.concatenate(gw8s, axis=0)                   # [2L*128, KO*H]
    gwh = np.concatenate(gwhs, axis=0)                   # [L*128, KO*H]
    gnb = np.ascontiguousarray(np.concatenate(gnbs, axis=1))
    ghb = np.ascontiguousarray(np.concatenate(ghbs, axis=1))
    ipw = in_proj_w.astype(np.float32)
    wq_ = _feature_major(ipw[:H].T).astype(BF16)
    wk_ = np.ascontiguousarray(
        ipw[H:2 * H].reshape(NH, P, H).transpose(1, 0, 2).reshape(P, NH * H)
    ).astype(BF16)
    wv_ = (_feature_major(ipw[2 * H:].T) * WS).astype(F8E4)
    qb_ = np.ascontiguousarray(
        (in_proj_b[:H] * QSCALE).reshape(KO, P).T.astype(np.float32))
    vb_ = np.ascontiguousarray(
        in_proj_b[2 * H:].reshape(NH, P).T.astype(np.float32))
    ow_ = _feature_major(out_w.T.astype(np.float32)).astype(BF16)
    ob_ = np.ascontiguousarray(out_b.reshape(KO, P).T.astype(np.float32))
    fcw_ = _feature_major(fc_w.T.astype(np.float32)).astype(BF16)
    fcb_ = np.ascontiguousarray(
        fc_b.reshape(O // P, P).T.astype(np.float32))
    idm_ = np.eye(P, dtype=np.float32)
    shared = dict(gw8=gw8, gwh=gwh, gnb=gnb, ghb=ghb, wq=wq_, wk=wk_, wv=wv_,
                  qb=qb_, vb=vb_, ow=ow_, ob=ob_, fcw=fcw_, fcb=fcb_,
                  idm=idm_)
    in_maps = []
    for c in range(NCORES):
        shard = x[c * BC:(c + 1) * BC]                   # [BC, S, H]
        xt = shard.transpose(2, 0, 1).reshape(H, BS)     # [H, BS]
        xt = _feature_major(xt.astype(np.float32))       # [128, KO*BS]
        in_maps.append(dict(shared, xT8=(xt * XS).astype(F8E4),
                            xTb=xt.astype(BF16)))
    return in_maps


def kernel(x, Wf, bf, Wi, bi, Wh, bh, in_proj_w, in_proj_b, out_w, out_b,
           fc_w, fc_b):
    from concourse.bass_utils import run_bass_kernel_spmd

    x, Wf, bf, Wi, bi, Wh, bh = (np.asarray(t) for t in
                                 (x, Wf, bf, Wi, bi, Wh, bh))
    in_proj_w, in_proj_b, out_w, out_b, fc_w, fc_b = (
        np.asarray(t) for t in (in_proj_w, in_proj_b, out_w, out_b,
                                fc_w, fc_b))
    if "nc" not in _CACHE:
        _CACHE["nc"] = _build_nc()
    nc = _CACHE["nc"]
    in_maps = _prep_inputs(x, Wf, bf, Wi, bi, Wh, bh, in_proj_w, in_proj_b,
                           out_w, out_b, fc_w, fc_b)
    res = run_bass_kernel_spmd(nc, in_maps, core_ids=list(range(NCORES)))
    _CACHE["last_results"] = res
    out = np.empty((B, O), np.float32)
    for c in range(NCORES):
        outT = res.results[c]["outT"]                    # [O, BC]
        for b in range(BC):
            out[c * BC + b] = outT[:, b]
    return out


# revision 18
# speedup vs baseline: 1.0197x; 1.0006x over previous
"""Trainium2 Bass kernel for DeepMinAttLSTM (4x minLSTM + MHSA + last-step FC).

Strategy:
  - Data-parallel over batch: 16 batches -> 8 cores x 2 batches.
  - Activations feature-major: X^T [H=1024 (8 tiles of 128), B*S=2048] bf16,
    plus an fp8 e4m3 copy (x*16) for the gate matmuls.
  - Per layer: f/i gate matmuls in fp8 with DoubleRow (2x PE throughput,
    W*256 / x*16 scaling, descale 1/4096 folded into the activation),
    h~ matmul in bf16. Gate math: f,i sigmoids on ACT, d=f+i on GpSimd
    (offload), r=1/d via DVE reciprocal_approx_fast, fp=f*r, and the
    input-gate branch uses ip = 1-fp exactly: add = h~' - fp*h~'.
  - Recurrence via DVE tensor_tensor_scan along time (fp32 state).
  - Attention collapses to the last query position; the full K matmul is
    eliminated via q~_bj = Wk_j^T q_bj so scores_bjk = q~_bj . h4_bk
    (K bias shifts all logits equally -> cancels in softmax). Scores, V,
    and the o-accumulation all run in fp8 DoubleRow.
  - Expected rel err ~1.2e-2 (fp8 gates dominate; threshold 2e-2).
"""

import math

import numpy as np
import ml_dtypes

BF16 = ml_dtypes.bfloat16
F8E4 = ml_dtypes.float8_e4m3

P = 128
H = 1024
S = 1024
B = 16
NCORES = 8
BC = B // NCORES          # batches per core
BS = BC * S               # 2048 free columns per core
KO = H // P               # 8 feature partition-tiles
NH = 8
DH = H // NH              # 128
O = 256
L = 4
QSCALE = 1.0 / math.sqrt(DH)
XS = 16.0                 # fp8 scale for activations
WS = 256.0                # fp8 scale for gate weights
ES = 64.0                 # fp8 scale for attention small tensors
GSC = 1.0 / (XS * WS)     # psum descale for fp8 gate matmuls

_CACHE = {}


def _build_nc():
    import concourse.mybir as mybir
    import concourse.tile as tile
    from concourse import bacc

    DT = mybir.dt.bfloat16
    F8 = mybir.dt.float8e4
    F32 = mybir.dt.float32
    AFT = mybir.ActivationFunctionType
    OP = mybir.AluOpType
    DR = mybir.MatmulPerfMode.DoubleRow

    nc = bacc.Bacc("TRN2", target_bir_lowering=False, debug=False,
                   num_devices=NCORES)

    xT8 = nc.dram_tensor("xT8", [P, KO * BS], F8, kind="ExternalInput").ap()
    xTb = nc.dram_tensor("xTb", [P, KO * BS], DT, kind="ExternalInput").ap()
    gw8 = nc.dram_tensor("gw8", [2 * L * P, KO * H], F8,
                         kind="ExternalInput").ap()
    gwh = nc.dram_tensor("gwh", [L * P, KO * H], DT, kind="ExternalInput").ap()
    gnb = nc.dram_tensor("gnb", [P, 2 * L * KO], F32,
                         kind="ExternalInput").ap()
    ghb = nc.dram_tensor("ghb", [P, L * KO], F32, kind="ExternalInput").ap()
    wq = nc.dram_tensor("wq", [P, KO * H], DT, kind="ExternalInput").ap()
    wk = nc.dram_tensor("wk", [P, NH * H], DT, kind="ExternalInput").ap()
    wv = nc.dram_tensor("wv", [P, KO * H], F8, kind="ExternalInput").ap()
    qb = nc.dram_tensor("qb", [P, KO], F32, kind="ExternalInput").ap()
    vb = nc.dram_tensor("vb", [P, NH], F32, kind="ExternalInput").ap()
    ow = nc.dram_tensor("ow", [P, KO * H], DT, kind="ExternalInput").ap()
    ob = nc.dram_tensor("ob", [P, KO], F32, kind="ExternalInput").ap()
    fcw = nc.dram_tensor("fcw", [P, KO * O], DT, kind="ExternalInput").ap()
    fcb = nc.dram_tensor("fcb", [P, O // P], F32, kind="ExternalInput").ap()
    idm = nc.dram_tensor("idm", [P, P], F32, kind="ExternalInput").ap()
    outT = nc.dram_tensor("outT", [O, BC], F32, kind="ExternalOutput").ap()

    with tile.TileContext(nc) as tc:
        with (
            tc.tile_pool(name="constp", bufs=1) as constp,
            tc.tile_pool(name="hbuf", bufs=2) as hp,
        ):
            gnb_sb = constp.tile([P, 2 * L * KO], F32)
            ghb_sb = constp.tile([P, L * KO], F32)
            wv_sb = constp.tile([P, KO * H], F8)
            nc.gpsimd.dma_start(gnb_sb[:], gnb[:])
            nc.gpsimd.dma_start(ghb_sb[:], ghb[:])

            x8p = tc.tile_pool(name="x8p", bufs=2)
            x8pool = x8p.__enter__()

            X = hp.tile([P, KO * BS], DT, tag="hbuf", name="xT_sb")
            X8 = x8pool.tile([P, KO * BS], F8, tag="x8", name="x8_0")
            xT8_v = xT8.rearrange("p (k m) -> p k m", k=KO)
            xTb_v = xTb.rearrange("p (k m) -> p k m", k=KO)
            X_v = X.rearrange("p (k m) -> p k m", k=KO)
            X8_v = X8.rearrange("p (k m) -> p k m", k=KO)
            for ch in range(4):
                m0 = ch * 512
                nc.sync.dma_start(X8_v[:, :, m0:m0 + 512],
                                  xT8_v[:, :, m0:m0 + 512])
                nc.scalar.dma_start(X_v[:, :, m0:m0 + 512],
                                    xTb_v[:, :, m0:m0 + 512])

            # ---------------- minLSTM layers ----------------
            with (
                tc.tile_pool(name="gw8p", bufs=2) as gw8p,
                tc.tile_pool(name="gwhp", bufs=2) as gwhp,
                tc.tile_pool(name="fpp", bufs=3) as fpp,
                tc.tile_pool(name="addp", bufs=3) as addp,
                tc.tile_pool(name="tmpp", bufs=3) as tmpp,
                tc.tile_pool(name="psA", bufs=6, space="PSUM") as psA,
            ):
                for l in range(L):
                    if l == 1:
                        nc.sync.dma_start(wv_sb[:], wv[:])
                    gwf = gw8p.tile([P, KO * H], F8, tag="gwf",
                                    name=f"gwf_{l}")
                    gwi = gw8p.tile([P, KO * H], F8, tag="gwi",
                                    name=f"gwi_{l}")
                    gwhb = gwhp.tile([P, KO * H], DT, tag="gwh",
                                     name=f"gwh_{l}")
                    gwf_v = gwf.rearrange("p (k h) -> p k h", k=KO)
                    gwi_v = gwi.rearrange("p (k h) -> p k h", k=KO)
                    gw8_v = gw8.rearrange("(g p) (k h) -> g p k h", p=P, k=KO)
                    # chunked loads so layer-0 matmuls start early
                    for kp in range(4):
                        nc.gpsimd.dma_start(
                            gwf_v[:, 2 * kp:2 * kp + 2, :],
                            gw8_v[2 * l, :, 2 * kp:2 * kp + 2, :])
                    for kp in range(4):
                        nc.gpsimd.dma_start(
                            gwi_v[:, 2 * kp:2 * kp + 2, :],
                            gw8_v[2 * l + 1, :, 2 * kp:2 * kp + 2, :])
                    for hf in range(2):
                        nc.gpsimd.dma_start(
                            gwhb[:, hf * 4 * H:(hf + 1) * 4 * H],
                            gwh[l * P:(l + 1) * P,
                                hf * 4 * H:(hf + 1) * 4 * H])

                    h_out = hp.tile([P, KO * BS], DT, tag="hbuf",
                                    name=f"h_{l}")
                    X8n = x8pool.tile([P, KO * BS], F8, tag="x8",
                                      name=f"x8_{l + 1}")
                    X8c_v = X8.rearrange("p (k m) -> p k m", k=KO)
                    if l == L - 1:
                        # ch-major on the last layer: all b=0 scans/quantizes
                        # finish by mid-layer so attention V can overlap the
                        # layer tail
                        units = [(no, ch) for ch in range(4)
                                 for no in range(KO)]
                    else:
                        units = [(no, ch) for no in range(KO)
                                 for ch in range(4)]
                    for no, ch in units:
                        if True:
                            m0 = ch * 512
                            b, half = ch // 2, ch % 2
                            base = no * BS + b * S
                            s0 = base + half * 512
                            psF = psA.tile([P, 512], F32, tag="ps", name="psF")
                            psI = psA.tile([P, 512], F32, tag="ps", name="psI")
                            psH = psA.tile([P, 512], F32, tag="ps", name="psH")
                            for kp in range(4):
                                nc.tensor.matmul(
                                    psF[:],
                                    gwf_v[:, 2 * kp:2 * kp + 2,
                                          no * P:(no + 1) * P],
                                    X8c_v[:, 2 * kp:2 * kp + 2, m0:m0 + 512],
                                    start=(kp == 0), stop=(kp == 3),
                                    perf_mode=DR)
                            for kp in range(4):
                                nc.tensor.matmul(
                                    psI[:],
                                    gwi_v[:, 2 * kp:2 * kp + 2,
                                          no * P:(no + 1) * P],
                                    X8c_v[:, 2 * kp:2 * kp + 2, m0:m0 + 512],
                                    start=(kp == 0), stop=(kp == 3),
                                    perf_mode=DR)
                            for ko in range(KO):
                                nc.tensor.matmul(
                                    psH[:],
                                    gwhb[:, ko * H + no * P:
                                         ko * H + (no + 1) * P],
                                    X[:, ko * BS + m0: ko * BS + m0 + 512],
                                    start=(ko == 0), stop=(ko == KO - 1))
                            bF = gnb_sb[:, (l * 2 + 0) * KO + no:
                                        (l * 2 + 0) * KO + no + 1]
                            bI = gnb_sb[:, (l * 2 + 1) * KO + no:
                                        (l * 2 + 1) * KO + no + 1]
                            bH = ghb_sb[:, l * KO + no: l * KO + no + 1]
                            f_t = tmpp.tile([P, 512], DT, tag="f", name="f_t")
                            i_t = tmpp.tile([P, 512], DT, tag="i", name="i_t")
                            d_t = tmpp.tile([P, 512], F32, tag="d", name="d_t")
                            r_t = tmpp.tile([P, 512], F32, tag="r", name="r_t")
                            ip_t = tmpp.tile([P, 512], DT, tag="ip",
                                             name="ip_t")
                            fp_t = fpp.tile([P, 512], DT, tag="fp",
                                            name="fp_t")
                            add_t = addp.tile([P, 512], DT, tag="add",
                                              name="add_t")
                            nc.scalar.activation(f_t[:], psF[:], AFT.Sigmoid,
                                                 bias=bF, scale=GSC)
                            nc.scalar.activation(i_t[:], psI[:], AFT.Sigmoid,
                                                 bias=bI, scale=GSC)
                            # d = f+i on GpSimd (DVE offload); fp = f/d;
                            # ip = 1-fp exactly; add = (zh+bh)*ip straight
                            # from PSUM (no h~ copy needed)
                            nc.gpsimd.tensor_add(d_t[:], f_t[:], i_t[:])
                            nc.vector.reciprocal_approx_fast(r_t[:], d_t[:])
                            nc.vector.tensor_mul(fp_t[:], f_t[:], r_t[:])
                            nc.vector.tensor_scalar(ip_t[:], fp_t[:], -1.0,
                                                    1.0, op0=OP.mult,
                                                    op1=OP.add)
                            nc.vector.scalar_tensor_tensor(
                                add_t[:], psH[:], bH, ip_t[:],
                                op0=OP.add, op1=OP.mult)
                            if half == 0:
                                nc.vector.tensor_tensor_scan(
                                    h_out[:, s0:s0 + 512],
                                    fp_t[:], add_t[:],
                                    initial=0.0, op0=OP.mult, op1=OP.add)
                            else:
                                nc.vector.tensor_tensor_scan(
                                    h_out[:, s0:s0 + 512],
                                    fp_t[:], add_t[:],
                                    initial=h_out[:, s0 - 1:s0],
                                    op0=OP.mult, op1=OP.add)
                            nc.scalar.activation(
                                X8n[:, s0:s0 + 512], h_out[:, s0:s0 + 512],
                                AFT.Copy, scale=XS)
                    X = h_out
                    X8 = X8n

            h4 = X
            h4_8 = X8
            h48_v = h4_8.rearrange("p (k m) -> p k m", k=KO)

            # ---------------- attention (last query position only) ----------
            with (
                tc.tile_pool(name="awp", bufs=1) as awp,
                tc.tile_pool(name="vp", bufs=1) as vp,
                tc.tile_pool(name="smallp", bufs=1) as smallp,
            ):
                wq_sb = awp.tile([P, KO * H], DT)
                nc.sync.dma_start(wq_sb[:], wq[:])
                wk_sb = awp.tile([P, NH * H], DT)
                nc.sync.dma_start(wk_sb[:], wk[:])
                wv_v = wv_sb.rearrange("p (k h) -> p k h", k=KO)
                ow_sb = awp.tile([P, KO * H], DT)
                nc.sync.dma_start(ow_sb[:], ow[:])
                fcw_sb = awp.tile([P, KO * O], DT)
                nc.sync.dma_start(fcw_sb[:], fcw[:])
                qb_sb = constp.tile([P, KO], F32)
                nc.gpsimd.dma_start(qb_sb[:], qb[:])
                vb_sb = constp.tile([P, NH], F32)
                nc.gpsimd.dma_start(vb_sb[:], vb[:])
                ob_sb = constp.tile([P, KO], F32)
                nc.gpsimd.dma_start(ob_sb[:], ob[:])
                fcb_sb = constp.tile([P, O // P], F32)
                nc.gpsimd.dma_start(fcb_sb[:], fcb[:])
                idm_sb = constp.tile([P, P], F32)
                nc.gpsimd.dma_start(idm_sb[:], idm[:])

                V_sb = vp.tile([P, BC * KO * H], F8, name="V_sb")
                V_v = V_sb.rearrange("p (b k h) -> p b k h", b=BC, k=KO)
                lastq = smallp.tile([P, KO * BC], DT)     # col = ko*2 + b
                q_sb = smallp.tile([P, KO * BC], DT)      # col = nt*2 + b
                qt_fm = smallp.tile([P, KO * NH * BC], F8)  # col=(dt*8+j)*2+b
                e_sb = smallp.tile([NH, BC * S], DT)      # col = b*S + s
                en_sb = smallp.tile([NH, BC * S], F32)
                acc = smallp.tile([NH, 2 * BC], F32)      # col = b*2 + ch
                den = smallp.tile([NH, BC], F32)
                rden = smallp.tile([NH, BC], F32)
                eT_sb = smallp.tile([P, BC * KO * NH], F8)  # col=(b*8+kt)*8+j
                eT_v = eT_sb.rearrange("p (b k j) -> p b k j", b=BC, k=KO)
                O_last = smallp.tile([P, NH * BC], DT)    # col = j*2 + b
                out_last = smallp.tile([P, KO * BC], DT)
                res_sb = smallp.tile([P, 2 * (O // P)], F32)

                # h4 columns at the last timestep: one strided copy
                h4_l = h4.rearrange("p (k b s) -> p k b s", k=KO, b=BC)
                lq_v = lastq.rearrange("p (k b) -> p k b", k=KO)
                nc.vector.tensor_copy(lq_v[:, :, :],
                                      h4_l[:, :, :, S - 1:S])

                with (
                    tc.tile_pool(name="psV", bufs=2, space="PSUM") as psV,
                    tc.tile_pool(name="psS", bufs=2, space="PSUM") as psS,
                    tc.tile_pool(name="psT", bufs=2, space="PSUM") as psT,
                    tc.tile_pool(name="psO", bufs=1, space="PSUM") as psO,
                ):
                    # V position-major [BS, H] via fp8 DoubleRow, stored fp8
                    for st in range(BC * KO):
                        b, si = st // KO, st % KO
                        m0 = b * S + si * P
                        for dch in range(2):
                            d0 = dch * 512
                            ps = psV.tile([P, 512], F32, tag="v", name="psv")
                            for kp in range(4):
                                nc.tensor.matmul(
                                    ps[:],
                                    h48_v[:, 2 * kp:2 * kp + 2, m0:m0 + P],
                                    wv_v[:, 2 * kp:2 * kp + 2, d0:d0 + 512],
                                    start=(kp == 0), stop=(kp == 3),
                                    perf_mode=DR)
                            nc.scalar.activation(
                                V_sb[:, st * H + d0: st * H + d0 + 512],
                                ps[:], AFT.Copy, scale=GSC * ES)

                    # q at the last position (scaled by QSCALE via bias+scale)
                    for nt in range(KO):
                        ps = psS.tile([P, BC], F32, tag="s", name="psq")
                        for ko in range(KO):
                            nc.tensor.matmul(
                                ps[:],
                                wq_sb[:, ko * H + nt * P:
                                      ko * H + (nt + 1) * P],
                                lastq[:, ko * BC:(ko + 1) * BC],
                                start=(ko == 0), stop=(ko == KO - 1))
                        nc.scalar.activation(
                            q_sb[:, nt * BC:(nt + 1) * BC], ps[:],
                            AFT.Identity, bias=qb_sb[:, nt:nt + 1],
                            scale=QSCALE)
                    # q~ feature-major fp8: q~[dt-tile, (j, b)] = Wk_j^T q_bj
                    for dt in range(KO):
                        ps = psS.tile([P, NH * BC], F32, tag="s", name="psqt")
                        for j in range(NH):
                            nc.tensor.matmul(
                                ps[:, j * BC:(j + 1) * BC],
                                wk_sb[:, j * H + dt * P: j * H + (dt + 1) * P],
                                q_sb[:, j * BC:(j + 1) * BC],
                                start=True, stop=True)
                        nc.scalar.activation(
                            qt_fm[:, dt * NH * BC:(dt + 1) * NH * BC], ps[:],
                            AFT.Copy, scale=ES)
                    # scores [8 heads, 512 pos] per (b, ch) via fp8 DR + exp
                    qt_v = qt_fm.rearrange("p (k j b) -> p k j b", k=KO, j=NH)
                    for b in range(BC):
                        for ch in range(2):
                            m0 = b * S + ch * 512
                            ps = psS.tile([NH, 512], F32, tag="s",
                                          name="pssc")
                            for kp in range(4):
                                nc.tensor.matmul(
                                    ps[:],
                                    qt_v[:, 2 * kp:2 * kp + 2, :, b:b + 1],
                                    h48_v[:, 2 * kp:2 * kp + 2, m0:m0 + 512],
                                    start=(kp == 0), stop=(kp == 3),
                                    perf_mode=DR)
                            nc.scalar.activation(
                                e_sb[:, m0:m0 + 512], ps[:], AFT.Exp,
                                scale=1.0 / (ES * XS),
                                accum_out=acc[:, b * 2 + ch:b * 2 + ch + 1])
                    # denominators and normalized weights (fp8, *ES)
                    acc_v = acc.rearrange("p (b c) -> p b c", b=BC)
                    nc.vector.tensor_add(den[:, :], acc_v[:, :, 0:1],
                                         acc_v[:, :, 1:2])
                    nc.vector.reciprocal_approx_fast(rden[:, :], den[:, :])
                    for b in range(BC):
                        nc.scalar.activation(
                            en_sb[:, b * S:(b + 1) * S],
                            e_sb[:, b * S:(b + 1) * S],
                            AFT.Copy, scale=rden[:, b:b + 1])
                    # transpose normalized weights to [pos, head], cast fp8
                    for b in range(BC):
                        for kt in range(KO):
                            pst = psT.tile([P, NH], F32, tag="t", name="pst")
                            nc.tensor.transpose(
                                pst[:],
                                en_sb[:, b * S + kt * P: b * S + (kt + 1) * P],
                                idm_sb[0:NH, 0:NH])
                            nc.scalar.activation(
                                eT_sb[:, (b * KO + kt) * NH:
                                      (b * KO + kt + 1) * NH],
                                pst[:], AFT.Copy, scale=ES)

                    # o at last position via fp8 DoubleRow over position tiles
                    ps_ob = psO.tile([P, NH * BC], F32, tag="o", name="ps_ob")
                    for b in range(BC):
                        for j in range(NH):
                            c = j * BC + b
                            for kp in range(4):
                                nc.tensor.matmul(
                                    ps_ob[:, c:c + 1],
                                    V_v[:, b, 2 * kp:2 * kp + 2,
                                        j * P:(j + 1) * P],
                                    eT_v[:, b, 2 * kp:2 * kp + 2, j:j + 1],
                                    start=(kp == 0), stop=(kp == 3),
                                    perf_mode=DR)
                    for j in range(NH):
                        nc.scalar.activation(
                            O_last[:, j * BC:(j + 1) * BC],
                            ps_ob[:, j * BC:(j + 1) * BC],
                            AFT.Identity, bias=vb_sb[:, j:j + 1],
                            scale=1.0 / (ES * ES))
                    # out projection + residual
                    for no in range(KO):
                        ps = psS.tile([P, BC], F32, tag="s", name="psp")
                        for ko in range(KO):
                            nc.tensor.matmul(
                                ps[:],
                                ow_sb[:, ko * H + no * P: ko * H + (no + 1) * P],
                                O_last[:, ko * BC:(ko + 1) * BC],
                                start=(ko == 0), stop=(ko == KO - 1))
                        nc.vector.scalar_tensor_tensor(
                            out_last[:, no * BC:(no + 1) * BC],
                            ps[:], ob_sb[:, no:no + 1],
                            lastq[:, no * BC:(no + 1) * BC],
                            op0=OP.add, op1=OP.add)
                    # final fc
                    for ot in range(O // P):
                        ps = psS.tile([P, BC], F32, tag="s", name="psf")
                        for ko in range(KO):
                            nc.tensor.matmul(
                                ps[:],
                                fcw_sb[:, ko * O + ot * P: ko * O + (ot + 1) * P],
                                out_last[:, ko * BC:(ko + 1) * BC],
                                start=(ko == 0), stop=(ko == KO - 1))
                        nc.scalar.activation(
                            res_sb[:, ot * BC:(ot + 1) * BC], ps[:],
                            AFT.Identity, bias=fcb_sb[:, ot:ot + 1])
                        nc.sync.dma_start(
                            outT[ot * P:(ot + 1) * P, :],
                            res_sb[:, ot * BC:(ot + 1) * BC])

            x8p.__exit__(None, None, None)

    nc.compile()
    return nc


def _feature_major(w_t):
    """[H_in, N] (already transposed weight) -> device layout [128, KO*N]."""
    hin, n = w_t.shape
    ko = hin // P
    return np.ascontiguousarray(
        w_t.reshape(ko, P, n).transpose(1, 0, 2).reshape(P, ko * n))


def _prep_inputs(x, Wf, bf, Wi, bi, Wh, bh, in_proj_w, in_proj_b, out_w,
                 out_b, fc_w, fc_b):
    gw8s, gwhs, gnbs, ghbs = [], [], [], []
    for l in range(L):
        for W, bias in ((Wf[l], bf[l]), (Wi[l], bi[l])):
            fm = _feature_major(W.T.astype(np.float32) * WS)
            gw8s.append(fm.astype(F8E4))
            gnbs.append(bias.reshape(KO, P).T.astype(np.float32))
        gwhs.append(_feature_major(Wh[l].T.astype(np.float32)).astype(BF16))
        ghbs.append(bh[l].reshape(KO, P).T.astype(np.float32))
    gw8 = np.concatenate(gw8s, axis=0)                   # [2L*128, KO*H]
    gwh = np.concatenate(gwhs, axis=0)                   # [L*128, KO*H]
    gnb = np.ascontiguousarray(np.concatenate(gnbs, axis=1))
    ghb = np.ascontiguousarray(np.concatenate(ghbs, axis=1))
    ipw = in_proj_w.astype(np.float32)
    wq_ = _feature_major(ipw[:H].T).astype(BF16)
    wk_ = np.ascontiguousarray(
        ipw[H:2 * H].reshape(NH, P, H).transpose(1, 0, 2).reshape(P, NH * H)
    ).astype(BF16)
    wv_ = (_feature_major(ipw[2 * H:].T) * WS).astype(F8E4)
    qb_ = np.ascontiguousarray(
        (in_proj_b[:H] * QSCALE).reshape(KO, P).T.astype(np.float32))
    vb_ = np.ascontiguousarray(
        in_proj_b[2 * H:].reshape(NH, P).T.astype(np.float32))
    ow_ = _feature_major(out_w.T.astype(np.float32)).astype(BF16)
    ob_ = np.ascontiguousarray(out_b.reshape(KO, P).T.astype(np.float32))
    fcw_ = _feature_major(fc_w.T.astype(np.float32)).astype(BF16)
    fcb_ = np.ascontiguousarray(
        fc_b.reshape(O // P, P).T.astype(np.float32))
    idm_ = np.eye(P, dtype=np.float32)
    shared = dict(gw8=gw8, gwh=gwh, gnb=gnb, ghb=ghb, wq=wq_, wk=wk_, wv=wv_,
                  qb=qb_, vb=vb_, ow=ow_, ob=ob_, fcw=fcw_, fcb=fcb_,
                  idm=idm_)
    in_maps = []
    for c in range(NCORES):
        shard = x[c * BC:(c + 1) * BC]                   # [BC, S, H]
        xt = shard.transpose(2, 0, 1).reshape(H, BS)     # [H, BS]
        xt = _feature_major(xt.astype(np.float32))       # [128, KO*BS]
        in_maps.append(dict(shared, xT8=(xt * XS).astype(F8E4),
                            xTb=xt.astype(BF16)))
    return in_maps


def kernel(x, Wf, bf, Wi, bi, Wh, bh, in_proj_w, in_proj_b, out_w, out_b,
           fc_w, fc_b):
    from concourse.bass_utils import run_bass_kernel_spmd

    x, Wf, bf, Wi, bi, Wh, bh = (np.asarray(t) for t in
                                 (x, Wf, bf, Wi, bi, Wh, bh))
    in_proj_w, in_proj_b, out_w, out_b, fc_w, fc_b = (
        np.asarray(t) for t in (in_proj_w, in_proj_b, out_w, out_b,
                                fc_w, fc_b))
    if "nc" not in _CACHE:
        _CACHE["nc"] = _build_nc()
    nc = _CACHE["nc"]
    in_maps = _prep_inputs(x, Wf, bf, Wi, bi, Wh, bh, in_proj_w, in_proj_b,
                           out_w, out_b, fc_w, fc_b)
    res = run_bass_kernel_spmd(nc, in_maps, core_ids=list(range(NCORES)))
    _CACHE["last_results"] = res
    out = np.empty((B, O), np.float32)
    for c in range(NCORES):
        outT = res.results[c]["outT"]                    # [O, BC]
        for b in range(BC):
            out[c * BC + b] = outT[:, b]
    return out
